# revision 1
# baseline (speedup 1.0000x reference)
"""JiT/DiT transformer block (adaLN + attention + SwiGLU) on 8 TRN2 NeuronCores.

Data-parallel over batch: core i computes batch element i end-to-end; no
collectives. Activations are kept "transposed" on device ([channel, seq]) so
per-channel modulation/bias are per-partition scalars; attention scores are
produced directly in [k, q] layout (softmax denominator via a ones-row
appended to V inside the AV matmul). Matmuls run bf16 with fp32 PSUM
accumulation; the residual stream stays fp32.
"""

import sys

sys.path.insert(0, "/opt/trn_rl_repo")

import numpy as np
import ml_dtypes

import concourse.bacc as bacc
import concourse.bass as bass
import concourse.mybir as mybir
from concourse.tile import TileContext
from concourse.bass_utils import run_bass_kernel_spmd

F32 = mybir.dt.float32
BF16 = mybir.dt.bfloat16
AF = mybir.ActivationFunctionType
ALU = mybir.AluOpType

B, S, D, H = 8, 1024, 1024, 16
HD = D // H  # 64
INNER = 2730
INNER_P = 2816  # 22*128
P = 128
NT = 8
NKT12 = INNER_P // P  # 22
EPS = 1e-6

_CACHE = {}


def _to_pmaj(v):
    return np.ascontiguousarray(v.reshape(-1, P).T)


def _rope_perm():
    ev = np.arange(0, HD, 2)
    od = np.arange(1, HD, 2)
    perm = np.concatenate([ev, od])
    partner = np.concatenate([od, ev])
    return perm, partner


def _prep_weights(inp):
    """Host-side layout/dtype prep (reordering/padding only, no math)."""
    perm, partner = _rope_perm()
    chperm = (np.arange(D).reshape(H, HD) [:, perm]).reshape(-1)

    w_qkv, b_qkv = inp["w_qkv"], inp["b_qkv"]
    wq = w_qkv[:, 0:D][:, chperm]
    wk = w_qkv[:, D : 2 * D][:, chperm]
    wv = w_qkv[:, 2 * D :]
    bq = b_qkv[0:D][chperm]
    bk = b_qkv[D : 2 * D][chperm]
    bv = b_qkv[2 * D :]
    wv_ext = np.zeros((D, H * 65), np.float32)
    bv_ext = np.zeros((H * 65,), np.float32)
    for h in range(H):
        wv_ext[:, h * 65 : h * 65 + 64] = wv[:, h * 64 : (h + 1) * 64]
        bv_ext[h * 65 : h * 65 + 64] = bv[h * 64 : (h + 1) * 64]
        bv_ext[h * 65 + 64] = 1.0
    wqkv_cat = np.concatenate([wq, wk, wv_ext], axis=1)  # [D, 3088]

    w12, b12 = inp["w12"], inp["b12"]
    w12p = np.zeros((D, 2 * INNER_P), np.float32)
    b12p = np.zeros((2 * INNER_P,), np.float32)
    w12p[:, :INNER] = w12[:, :INNER]
    w12p[:, INNER_P : INNER_P + INNER] = w12[:, INNER:]
    b12p[:INNER] = b12[:INNER]
    b12p[INNER_P : INNER_P + INNER] = b12[INNER:]
    w3p = np.zeros((INNER_P, D), np.float32)
    w3p[:INNER] = inp["w3"]

    # rope tiles [128, S]: two stacked 64-row head-local blocks
    sign = np.where(np.arange(HD) < HD // 2, -1.0, 1.0).astype(np.float32)
    cos, sin = inp["rope_cos"], inp["rope_sin"]

    def rope_tiles(scale_vec):
        c64 = cos[:, perm].T * scale_vec[perm][:, None]
        s64 = (sin[:, perm].T * sign[:, None]) * scale_vec[partner][:, None]
        return (
            np.concatenate([c64, c64], 0).astype(np.float32),
            np.concatenate([s64, s64], 0).astype(np.float32),
        )

    cq, sq = rope_tiles(inp["qn_scale"])
    ck, sk = rope_tiles(inp["kn_scale"])

    E2 = np.zeros((2, P), np.float32)
    E2[0, 0:64] = 1.0
    E2[1, 64:128] = 1.0
    e65 = np.zeros((65, 64), np.float32)
    e65[64, :] = 1.0
    bo2 = np.zeros((P, 2), np.float32)
    bo2[0:64, 0] = 1.0
    bo2[64:128, 1] = 1.0

    bqk_T = np.stack(
        [bq.reshape(NT, P)[m] for m in range(NT)]
        + [bk.reshape(NT, P)[m] for m in range(NT)],
        axis=1,
    )

    return {
        "wqkv": wqkv_cat, "wproj": inp["w_proj"], "w12p": w12p, "w3p": w3p,
        "wada": inp["w_ada"], "bqk_T": bqk_T, "bv_ext": bv_ext[None, :],
        "b12T": _to_pmaj(b12p), "bprojT": _to_pmaj(inp["b_proj"]),
        "b3T": _to_pmaj(inp["b3"]), "n1T": _to_pmaj(inp["norm1_scale"]),
        "n2T": _to_pmaj(inp["norm2_scale"]), "b_ada": inp["b_ada"][None, :],
        "E2": E2, "e65": e65, "bo2": bo2, "ones1": np.ones((1, P), np.float32),
        "ident": np.eye(P, dtype=np.float32),
        "cos2q": cq, "sin2q": sq, "cos2k": ck, "sin2k": sk,
    }


BF16_NAMES = {
    "wqkv", "wproj", "w12p", "w3p", "wada", "bv_ext", "E2", "e65", "bo2", "ones1",
    "cos2q", "sin2q", "cos2k", "sin2k",
}


def build_bass():
    nc = bacc.Bacc("TRN2", target_bir_lowering=False, debug=False, num_devices=8)

    def par(name, shape, dt, out=False):
        return nc.declare_dram_parameter(name, list(shape), dt, isOutput=out)

    d = {
        "x": par("x", [S, D], F32),
        "cT": par("cT", [P, NT], F32),
        "wqkv": par("wqkv", [D, 2 * D + H * 65], BF16),
        "wproj": par("wproj", [D, D], BF16),
        "w12p": par("w12p", [D, 2 * INNER_P], BF16),
        "w3p": par("w3p", [INNER_P, D], BF16),
        "wada": par("wada", [D, 6 * D], BF16),
        "bqk_T": par("bqk_T", [P, 16], F32),
        "bv_ext": par("bv_ext", [1, H * 65], BF16),
        "b12T": par("b12T", [P, 2 * NKT12], F32),
        "bprojT": par("bprojT", [P, NT], F32),
        "b3T": par("b3T", [P, NT], F32),
        "n1T": par("n1T", [P, NT], F32),
        "n2T": par("n2T", [P, NT], F32),
        "b_ada": par("b_ada", [1, 6 * D], F32),
        "E2": par("E2", [2, P], BF16),
        "e65": par("e65", [65, 64], BF16),
        "bo2": par("bo2", [P, 2], BF16),
        "ones1": par("ones1", [1, P], BF16),
        "ident": par("ident", [P, P], F32),
        "cos2q": par("cos2q", [P, S], BF16),
        "sin2q": par("sin2q", [P, S], BF16),
        "cos2k": par("cos2k", [P, S], BF16),
        "sin2k": par("sin2k", [P, S], BF16),
        "out": par("out", [S, D], F32, out=True),
    }
    mods_dram = nc.dram_tensor("mods_scratch", [1, 6 * D], F32)
    kss_dram = nc.dram_tensor("kss_scratch", [H, S], F32)

    with TileContext(nc) as tc:
        _body(nc, tc, d, mods_dram, kss_dram)
    nc.compile()
    return nc


def _body(nc, tc, d, mods_dram, kss_dram):
    from contextlib import ExitStack

    with ExitStack() as ctx:
        const = ctx.enter_context(tc.tile_pool(name="const", bufs=1))
        persist = ctx.enter_context(tc.tile_pool(name="persist", bufs=1))
        small = ctx.enter_context(tc.tile_pool(name="small", bufs=1))
        scratch = ctx.enter_context(tc.tile_pool(name="scratch", bufs=2))
        psum = ctx.enter_context(tc.tile_pool(name="psum", bufs=6, space="PSUM"))

        def load_const(key, shape, dt, pool=None):
            t = (pool or const).tile(list(shape), dt, tag=key, name=key + "_sb")
            nc.sync.dma_start(out=t[:], in_=d[key][:])
            return t

        cT = load_const("cT", [P, NT], F32)
        bqkT = load_const("bqk_T", [P, 16], F32)
        bv = load_const("bv_ext", [1, H * 65], BF16)
        b12T = load_const("b12T", [P, 2 * NKT12], F32)
        bprojT = load_const("bprojT", [P, NT], F32)
        b3T = load_const("b3T", [P, NT], F32)
        n1T = load_const("n1T", [P, NT], F32)
        n2T = load_const("n2T", [P, NT], F32)
        bo2 = load_const("bo2", [P, 2], BF16)
        e65 = load_const("e65", [65, 64], BF16)
        ones1 = load_const("ones1", [1, P], BF16)
        ident = load_const("ident", [P, P], F32)
        ones128 = const.tile([P, P], BF16, tag="ones128", name="ones128")
        nc.vector.memset(ones128[:], 1.0)
        eps1 = const.tile([P, 1], F32, tag="eps1", name="eps1")
        nc.vector.memset(eps1[:], EPS)
        epsk = const.tile([P, 1], F32, tag="epsk", name="epsk")
        nc.vector.memset(epsk[:], HD * EPS)

        # residual stream lives here, updated in place
        xT = persist.tile([P, NT, S], F32, tag="bigf32", name="xT")
        invb = persist.tile([P, S], F32, tag="invb", name="invb")
        invrk8 = small.tile([P, NT, H], F32, name="invrk8", padded_shape=[P, NT, H + 1])

        def rms_invb(zT):
            # invb[:, ch*512:...] = 1/sqrt(mean_d z^2 + eps) (rows identical)
            for ch in range(2):
                ms = None
                for dt in range(NT):
                    sq = scratch.tile([P, 512], BF16, tag="sqd", name="sqd")
                    nc.vector.tensor_mul(
                        sq[:],
                        zT[:, dt, ch * 512 : (ch + 1) * 512],
                        zT[:, dt, ch * 512 : (ch + 1) * 512],
                    )
                    if dt == 0:
                        ms = psum.tile([P, 512], F32, tag="ps", name="ps_ms")
                    nc.tensor.matmul(
                        ms[:], ones128[:], sq[:],
                        start=(dt == 0), stop=(dt == NT - 1),
                    )
                rms = scratch.tile([P, 512], F32, tag="rms", name="rms")
                nc.scalar.activation(rms[:], ms[:], AF.Sqrt, bias=eps1[:], scale=1.0 / D)
                nc.vector.reciprocal_approx_fast(
                    invb[:, ch * 512 : (ch + 1) * 512], rms[:]
                )

        def modulate(zT, dstT, aa, sh):
            for dt in range(NT):
                tmp = scratch.tile([P, S], F32, tag="htmp", name="htmp")
                nc.vector.tensor_mul(tmp[:], zT[:, dt, :], invb[:])
                nc.vector.tensor_scalar(
                    dstT[:, dt, :], tmp[:], aa[:, dt : dt + 1], sh[:, dt : dt + 1],
                    op0=ALU.mult, op1=ALU.add,
                )

        # ======= Phases B-E =======
        with ExitStack() as actx:
            ho = actx.enter_context(tc.tile_pool(name="ho", bufs=1))
            hT = ho.tile([P, NT, S], BF16, tag="hT", name="hT")
            ohat = ho.tile([P, NT, S], BF16, tag="ohat", name="ohat")

            # ---- Phase B ----
            with tc.tile_pool(name="xin_pool", bufs=3) as xin_pool:
                for st in range(NT):
                    xin = xin_pool.tile([P, D], F32, tag="xin", name="xin")
                    nc.sync.dma_start(out=xin[:], in_=d["x"][st * P : (st + 1) * P, :])
                    for g4 in range(2):
                        pt = psum.tile([P, 512], F32, tag="ps", name="ps_tr")
                        for j in range(4):
                            dt = g4 * 4 + j
                            nc.tensor.transpose(
                                pt[:, j * P : (j + 1) * P],
                                xin[:, dt * P : (dt + 1) * P],
                                ident[:],
                            )
                        for j in range(4):
                            dt = g4 * 4 + j
                            nc.scalar.activation(
                                xT[:, dt, st * P : (st + 1) * P],
                                pt[:, j * P : (j + 1) * P],
                                AF.Copy,
                            )

            rms_invb(xT)

            # ============ Phase A: mods ============
            cT_silu = small.tile([P, NT], F32, name="cT_silu")
            nc.scalar.activation(cT_silu[:], cT[:], AF.Silu)
            cT_bf = small.tile([P, NT], BF16, name="cT_bf")
            nc.vector.tensor_copy(cT_bf[:], cT_silu[:])

            with tc.tile_pool(name="ada_sc", bufs=2) as ada_sc, tc.tile_pool(
                name="wada_pool", bufs=2
            ) as wada_pool:
                for n in range(12):
                    ps = psum.tile([1, 512], F32, tag="ps", name="ps_ada")
                    wt = wada_pool.tile([P, NT, 512], BF16, tag="wada", name="wada_t")
                    nc.sync.dma_start(
                        out=wt[:],
                        in_=d["wada"][:, n * 512 : (n + 1) * 512].rearrange(
                            "(kt p) c -> p kt c", p=P
                        ),
                    )
                    for kt in range(NT):
                        nc.tensor.matmul(
                            ps[:], cT_bf[:, kt : kt + 1], wt[:, kt, :],
                            start=(kt == 0), stop=(kt == NT - 1),
                        )
                    bch = ada_sc.tile([1, 512], F32, tag="bch", name="bada_ch")
                    nc.sync.dma_start(out=bch[:], in_=d["b_ada"][:, n * 512 : (n + 1) * 512])
                    mch = ada_sc.tile([1, 512], F32, tag="mch", name="mods_ch")
                    nc.vector.tensor_add(mch[:], ps[:], bch[:])
                    nc.sync.dma_start(
                        out=mods_dram[:, n * 512 : (n + 1) * 512], in_=mch[:]
                    )
            modsT = small.tile([P, 48], F32, name="modsT")
            nc.sync.dma_start(
                out=modsT[:], in_=mods_dram.ap()[0, :].rearrange("(t p) -> p t", p=P)
            )
            a1 = small.tile([P, NT], F32, name="a1")
            nc.vector.tensor_scalar_add(a1[:], modsT[:, 8:16], 1.0)
            nc.vector.tensor_mul(a1[:], a1[:], n1T[:])
            sh1 = modsT[:, 0:8]
            g1 = modsT[:, 16:24]
            g1b = small.tile([P, NT], F32, name="g1b")
            nc.vector.tensor_mul(g1b[:], g1, bprojT[:])
            a2 = small.tile([P, NT], F32, name="a2")
            nc.vector.tensor_scalar_add(a2[:], modsT[:, 32:40], 1.0)
            nc.vector.tensor_mul(a2[:], a2[:], n2T[:])
            sh2 = modsT[:, 24:32]
            g2 = modsT[:, 40:48]
            g2b3 = small.tile([P, NT], F32, name="g2b3")
            nc.vector.tensor_mul(g2b3[:], g2, b3T[:])


            modulate(xT, hT, a1, sh1)

            # ---- Phases C + D in a scoped block ----
            with ExitStack() as cctx:
                qk = cctx.enter_context(tc.tile_pool(name="qk", bufs=1))
                qhat = qk.tile([P, NT, S], BF16, tag="qhat", name="qhat")
                khat = qk.tile([P, NT, S], BF16, tag="khat", name="khat")
                v_sb = qk.tile([P, NT, H * 65], BF16, tag="v", name="v_sb")

                with ExitStack() as qctx:
                    ropec = qctx.enter_context(tc.tile_pool(name="ropec", bufs=1))
                    qkn = qctx.enter_context(tc.tile_pool(name="qkn", bufs=1))
                    wqk_pool = qctx.enter_context(tc.tile_pool(name="wqk_pool", bufs=3))
                    rope_sc = qctx.enter_context(tc.tile_pool(name="rope_sc", bufs=2))

                    cos2q = load_const("cos2q", [P, S], BF16, pool=ropec)
                    sin2q = load_const("sin2q", [P, S], BF16, pool=ropec)
                    cos2k = load_const("cos2k", [P, S], BF16, pool=ropec)
                    sin2k = load_const("sin2k", [P, S], BF16, pool=ropec)
                    E2 = load_const("E2", [2, P], BF16, pool=ropec)

                    for m in range(16):
                        isq = m < NT
                        mk = m if isq else m - NT
                        wt = wqk_pool.tile([P, NT, P], BF16, tag="wqk", name="wqk_t")
                        nc.sync.dma_start(
                            out=wt[:],
                            in_=d["wqkv"][:, m * P : (m + 1) * P].rearrange(
                                "(kt p) c -> p kt c", p=P
                            ),
                        )
                        raw = rope_sc.tile([P, S], BF16, tag="raw", name="qk_raw")
                        for sch in range(2):
                            ps = psum.tile([P, 512], F32, tag="ps", name="ps_qkv")
                            for kt in range(NT):
                                nc.tensor.matmul(
                                    ps[:], wt[:, kt, :],
                                    hT[:, kt, sch * 512 : (sch + 1) * 512],
                                    start=(kt == 0), stop=(kt == NT - 1),
                                )
                            nc.vector.tensor_scalar_add(
                                raw[:, sch * 512 : (sch + 1) * 512], ps[:],
                                bqkT[:, m : m + 1],
                            )
                            sqs = scratch.tile([P, 512], BF16, tag="sqd", name="sqs")
                            nc.vector.tensor_mul(
                                sqs[:],
                                raw[:, sch * 512 : (sch + 1) * 512],
                                raw[:, sch * 512 : (sch + 1) * 512],
                            )
                            ss = psum.tile([2, 512], F32, tag="ps", name="ps_ss")
                            nc.tensor.matmul(ss[:], bo2[:], sqs[:], start=True, stop=True)
                            if isq:
                                if sch == 0:
                                    qt = qkn.tile(
                                        [2, S], F32, tag="qstage", name="qstage", bufs=2
                                    )
                                nc.scalar.activation(
                                    qt[:, sch * 512 : (sch + 1) * 512],
                                    ss[:], AF.Copy,
                                )
                            else:
                                if sch == 0:
                                    kstage = qkn.tile(
                                        [2, S], F32, tag="kstage", name="kstage", bufs=2
                                    )
                                nc.scalar.activation(
                                    kstage[:, sch * 512 : (sch + 1) * 512], ss[:], AF.Copy
                                )
                                nc.sync.dma_start(
                                    out=kss_dram[
                                        2 * mk : 2 * mk + 2,
                                        sch * 512 : (sch + 1) * 512,
                                    ],
                                    in_=kstage[:, sch * 512 : (sch + 1) * 512],
                                )
                        rot = rope_sc.tile([P, S], BF16, tag="rot", name="rot", bufs=2)
                        for blk in range(4):
                            b0 = blk * 32
                            srcb = b0 + (32 if blk % 2 == 0 else -32)
                            nc.gpsimd.dma_start(
                                out=rot[b0 : b0 + 32, :], in_=raw[srcb : srcb + 32, :]
                            )
                        t1 = rope_sc.tile([P, S], BF16, tag="t1", name="rope_t1", bufs=2)
                        t2 = rope_sc.tile([P, S], BF16, tag="t2", name="rope_t2", bufs=2)
                        nc.vector.tensor_mul(t1[:], raw[:], cos2q[:] if isq else cos2k[:])
                        nc.vector.tensor_mul(t2[:], rot[:], sin2q[:] if isq else sin2k[:])
                        nc.vector.tensor_add(
                            (qhat if isq else khat)[:, mk, :], t1[:], t2[:]
                        )
                        if isq:
                            # inverse-rms of this q pair, folded into qhat now
                            nc.scalar.activation(
                                qt[:], qt[:], AF.Sqrt, bias=eps1[0:2, :],
                                scale=1.0 / HD,
                            )
                            nc.vector.reciprocal_approx_fast(qt[:], qt[:])
                            qbf = qkn.tile([2, S], BF16, tag="qbf", name="qbf", bufs=2)
                            nc.vector.tensor_copy(qbf[:], qt[:])
                            for sch in range(2):
                                pe = psum.tile([P, 512], F32, tag="ps", name="ps_erq")
                                nc.tensor.matmul(
                                    pe[:], E2[:],
                                    qbf[:, sch * 512 : (sch + 1) * 512],
                                    start=True, stop=True,
                                )
                                nc.vector.tensor_mul(
                                    qhat[:, mk, sch * 512 : (sch + 1) * 512],
                                    qhat[:, mk, sch * 512 : (sch + 1) * 512], pe[:],
                                )

                    # q inverse-rms per m-tile pair
                    kssT = qkn.tile([P, NT, H], F32, name="kssT", padded_shape=[P, NT, H + 1])
                    for kt in range(NT):
                        nc.sync.dma_start(
                            out=kssT[:, kt, :],
                            in_=kss_dram.ap()[:, kt * P : (kt + 1) * P].rearrange(
                                "h p -> p h"
                            ),
                        )
                    for kt in range(NT):
                        nc.scalar.activation(
                            kssT[:, kt, :], kssT[:, kt, :], AF.Sqrt,
                            bias=epsk[:], scale=1.0,
                        )
                        nc.vector.reciprocal_approx_fast(
                            invrk8[:, kt, :], kssT[:, kt, :]
                        )

                    # q inverse-rms handled inline above

                    # v
                    with tc.tile_pool(name="wv_pool", bufs=2) as wv_pool:
                        for nch in range(4):
                            c0 = nch * 260
                            wt = wv_pool.tile([P, NT, 260], BF16, tag="wv", name="wv_t")
                            nc.sync.dma_start(
                                out=wt[:],
                                in_=d["wqkv"][
                                    :, 2 * D + c0 : 2 * D + c0 + 260
                                ].rearrange("(kt p) c -> p kt c", p=P),
                            )
                            for st in range(NT):
                                ps = psum.tile([P, 260], F32, tag="ps", name="ps_v")
                                for kt in range(NT):
                                    nc.tensor.matmul(
                                        ps[:], hT[:, kt, st * P : (st + 1) * P],
                                        wt[:, kt, :],
                                        start=(kt == 0), stop=False,
                                    )
                                nc.tensor.matmul(
                                    ps[:], ones1[:], bv[:, c0 : c0 + 260],
                                    start=False, stop=True,
                                )
                                nc.vector.tensor_copy(
                                    v_sb[:, st, c0 : c0 + 260], ps[:]
                                )

                # ---- Phase D: attention ----
                with tc.tile_pool(name="ppool", bufs=3) as ppool, tc.tile_pool(
                    name="avp", bufs=2, space="PSUM"
                ) as avp, tc.tile_pool(name="att_sc", bufs=2) as att_sc:

                    def qk_exp(h, qch):
                        mk, hh = h // 2, h % 2
                        rb = 64 * hh
                        pT = ppool.tile([P, NT, 512], BF16, tag="pT", name="pT")
                        for kt in range(NT):
                            ps_s = psum.tile([P, 512], F32, tag="ps", name="ps_s")
                            nc.tensor.matmul(
                                ps_s[:],
                                khat[rb : rb + 64, mk, kt * P : (kt + 1) * P],
                                qhat[rb : rb + 64, mk, qch * 512 : (qch + 1) * 512],
                                start=True, stop=True,
                            )
                            nc.scalar.activation(
                                pT[:, kt, :], ps_s[:], AF.Exp,
                                scale=invrk8[:, kt, h : h + 1],
                            )
                        return pT

                    def av_div(h, qch, pT):
                        mk, hh = h // 2, h % 2
                        rb = 64 * hh
                        ps_av = avp.tile([65, 512], F32, tag="ps_av", name="ps_av")
                        for kt in range(NT):
                            nc.tensor.matmul(
                                ps_av[:], v_sb[:, kt, h * 65 : h * 65 + 65],
                                pT[:, kt, :],
                                start=(kt == 0), stop=(kt == NT - 1),
                            )
                        o65 = att_sc.tile([65, 512], F32, tag="o65", name="o65")
                        nc.vector.tensor_copy(o65[:], ps_av[:])
                        o65b = att_sc.tile([65, 512], BF16, tag="o65b", name="o65b")
                        nc.vector.tensor_copy(o65b[:], o65[:])
                        pb = psum.tile([64, 512], F32, tag="ps", name="ps_bc")
                        nc.tensor.matmul(pb[:], e65[:], o65b[:], start=True, stop=True)
                        rb64 = att_sc.tile([64, 512], F32, tag="rb64", name="rb64")
                        nc.vector.reciprocal_approx_fast(rb64[:], pb[:])
                        ob = att_sc.tile([64, 512], BF16, tag="ob", name="ob")
                        nc.vector.tensor_mul(ob[:], o65[0:64, :], rb64[:])
                        nc.sync.dma_start(
                            out=ohat[rb : rb + 64, mk, qch * 512 : (qch + 1) * 512],
                            in_=ob[:],
                        )

                    prev = None
                    for h in range(H):
                        for qch in range(2):
                            pT = qk_exp(h, qch)
                            if prev is not None:
                                av_div(*prev)
                            prev = (h, qch, pT)
                    av_div(*prev)

            # ---- Phase E: proj + residual 1 (in place on xT) ----
            with tc.tile_pool(name="wproj_pool", bufs=3) as wproj_pool:
                for dt in range(NT):
                    wt = wproj_pool.tile([P, NT, P], BF16, tag="wproj", name="wproj_t")
                    nc.sync.dma_start(
                        out=wt[:],
                        in_=d["wproj"][:, dt * P : (dt + 1) * P].rearrange(
                            "(kt p) c -> p kt c", p=P
                        ),
                    )
                    for qch in range(2):
                        ps = psum.tile([P, 512], F32, tag="ps", name="ps_proj")
                        for kt in range(NT):
                            nc.tensor.matmul(
                                ps[:], wt[:, kt, :],
                                ohat[:, kt, qch * 512 : (qch + 1) * 512],
                                start=(kt == 0), stop=(kt == NT - 1),
                            )
                        nc.vector.affine_then_add(
                            xT[:, dt, qch * 512 : (qch + 1) * 512],
                            ps[:], xT[:, dt, qch * 512 : (qch + 1) * 512],
                            scale=g1[:, dt : dt + 1], bias=g1b[:, dt : dt + 1],
                        )

        # ======= Phases F-H =======
        with ExitStack() as mctx:
            mlp = mctx.enter_context(tc.tile_pool(name="mlp", bufs=1))

            rms_invb(xT)

            # ============ Phase A: mods ============
            cT_silu = small.tile([P, NT], F32, name="cT_silu")
            nc.scalar.activation(cT_silu[:], cT[:], AF.Silu)
            cT_bf = small.tile([P, NT], BF16, name="cT_bf")
            nc.vector.tensor_copy(cT_bf[:], cT_silu[:])

            with tc.tile_pool(name="ada_sc", bufs=2) as ada_sc, tc.tile_pool(
                name="wada_pool", bufs=2
            ) as wada_pool:
                for n in range(12):
                    ps = psum.tile([1, 512], F32, tag="ps", name="ps_ada")
                    wt = wada_pool.tile([P, NT, 512], BF16, tag="wada", name="wada_t")
                    nc.sync.dma_start(
                        out=wt[:],
                        in_=d["wada"][:, n * 512 : (n + 1) * 512].rearrange(
                            "(kt p) c -> p kt c", p=P
                        ),
                    )
                    for kt in range(NT):
                        nc.tensor.matmul(
                            ps[:], cT_bf[:, kt : kt + 1], wt[:, kt, :],
                            start=(kt == 0), stop=(kt == NT - 1),
                        )
                    bch = ada_sc.tile([1, 512], F32, tag="bch", name="bada_ch")
                    nc.sync.dma_start(out=bch[:], in_=d["b_ada"][:, n * 512 : (n + 1) * 512])
                    mch = ada_sc.tile([1, 512], F32, tag="mch", name="mods_ch")
                    nc.vector.tensor_add(mch[:], ps[:], bch[:])
                    nc.sync.dma_start(
                        out=mods_dram[:, n * 512 : (n + 1) * 512], in_=mch[:]
                    )
            modsT = small.tile([P, 48], F32, name="modsT")
            nc.sync.dma_start(
                out=modsT[:], in_=mods_dram.ap()[0, :].rearrange("(t p) -> p t", p=P)
            )
            a1 = small.tile([P, NT], F32, name="a1")
            nc.vector.tensor_scalar_add(a1[:], modsT[:, 8:16], 1.0)
            nc.vector.tensor_mul(a1[:], a1[:], n1T[:])
            sh1 = modsT[:, 0:8]
            g1 = modsT[:, 16:24]
            g1b = small.tile([P, NT], F32, name="g1b")
            nc.vector.tensor_mul(g1b[:], g1, bprojT[:])
            a2 = small.tile([P, NT], F32, name="a2")
            nc.vector.tensor_scalar_add(a2[:], modsT[:, 32:40], 1.0)
            nc.vector.tensor_mul(a2[:], a2[:], n2T[:])
            sh2 = modsT[:, 24:32]
            g2 = modsT[:, 40:48]
            g2b3 = small.tile([P, NT], F32, name="g2b3")
            nc.vector.tensor_mul(g2b3[:], g2, b3T[:])


            h2T = mlp.tile([P, NT, S], BF16, tag="h2T", name="h2T")
            modulate(xT, h2T, a2, sh2)

            gg = mlp.tile([P, NKT12, S], BF16, tag="gg", name="gg")
            with tc.tile_pool(name="w12_pool", bufs=3) as w12_pool, tc.tile_pool(
                name="mlp_sc", bufs=2
            ) as mlp_sc:
                for j in range(NKT12):
                    outs = []
                    for part in range(2):
                        m = j + part * NKT12
                        wt = w12_pool.tile([P, NT, P], BF16, tag="w12", name="w12_t")
                        nc.sync.dma_start(
                            out=wt[:],
                            in_=d["w12p"][:, m * P : (m + 1) * P].rearrange(
                                "(kt p) c -> p kt c", p=P
                            ),
                        )
                        o = mlp_sc.tile([P, S], BF16, tag=f"mlp{part}", name=f"mlp{part}")
                        for sch in range(2):
                            ps = psum.tile([P, 512], F32, tag="ps", name="ps_mlp")
                            for kt in range(NT):
                                nc.tensor.matmul(
                                    ps[:], wt[:, kt, :],
                                    h2T[:, kt, sch * 512 : (sch + 1) * 512],
                                    start=(kt == 0), stop=(kt == NT - 1),
                                )
                            nc.scalar.activation(
                                o[:, sch * 512 : (sch + 1) * 512], ps[:],
                                AF.Silu if part == 0 else AF.Identity,
                                bias=b12T[:, m : m + 1],
                            )
                        outs.append(o)
                    nc.vector.tensor_mul(gg[:, j, :], outs[0][:], outs[1][:])

            # w3 + residual 2 (in place on xT)
            with tc.tile_pool(name="w3_pool", bufs=2) as w3_pool:
                for dt in range(NT):
                    wt = w3_pool.tile([P, NKT12, P], BF16, tag="w3", name="w3_t")
                    nc.sync.dma_start(
                        out=wt[:],
                        in_=d["w3p"][:, dt * P : (dt + 1) * P].rearrange(
                            "(kt p) c -> p kt c", p=P
                        ),
                    )
                    for qch in range(2):
                        ps = psum.tile([P, 512], F32, tag="ps", name="ps_w3")
                        for kt in range(NKT12):
                            nc.tensor.matmul(
                                ps[:], wt[:, kt, :],
                                gg[:, kt, qch * 512 : (qch + 1) * 512],
                                start=(kt == 0), stop=(kt == NKT12 - 1),
                            )
                        nc.vector.affine_then_add(
                            xT[:, dt, qch * 512 : (qch + 1) * 512],
                            ps[:], xT[:, dt, qch * 512 : (qch + 1) * 512],
                            scale=g2[:, dt : dt + 1], bias=g2b3[:, dt : dt + 1],
                        )

            # ---- Phase H ----
            with tc.tile_pool(name="yout", bufs=3) as ypool:
                for st in range(NT):
                    y = ypool.tile([P, D], F32, tag="y", name="y")
                    for g4 in range(2):
                        pt = psum.tile([P, 512], F32, tag="ps", name="ps_tr2")
                        for j in range(4):
                            dt = g4 * 4 + j
                            nc.tensor.transpose(
                                pt[:, j * P : (j + 1) * P],
                                xT[:, dt, st * P : (st + 1) * P],
                                ident[:],
                            )
                        for j in range(4):
                            dt = g4 * 4 + j
                            nc.scalar.activation(
                                y[:, dt * P : (dt + 1) * P],
                                pt[:, j * P : (j + 1) * P],
                                AF.Copy,
                            )
                    nc.sync.dma_start(out=d["out"][st * P : (st + 1) * P, :], in_=y[:])


def kernel(**inputs):
    inputs = {k: np.asarray(v) for k, v in inputs.items()}
    if "nc" not in _CACHE:
        _CACHE["nc"] = build_bass()
    nc = _CACHE["nc"]

    consts = _prep_weights(inputs)
    base = {}
    for k, v in consts.items():
        if k in BF16_NAMES:
            base[k] = np.ascontiguousarray(v).astype(ml_dtypes.bfloat16)
        else:
            base[k] = np.ascontiguousarray(v).astype(np.float32)

    in_maps = []
    for core in range(B):
        m = dict(base)
        m["x"] = np.ascontiguousarray(inputs["x"][core]).astype(np.float32)
        m["cT"] = _to_pmaj(inputs["c"][core]).astype(np.float32)
        in_maps.append(m)

    res = run_bass_kernel_spmd(
        nc, in_maps, core_ids=list(range(B)), **_CACHE.get("run_kwargs", {})
    )
    _CACHE["last_results"] = res
    return np.stack([res.results[i]["out"] for i in range(B)], axis=0)


if __name__ == "__main__":
    build_bass()
    print("built ok")



# revision 11
# speedup vs baseline: 1.4665x; 1.4665x over previous
"""JiT/DiT transformer block (adaLN + attention + SwiGLU) on 8 TRN2 NeuronCores.

Data-parallel over batch: core i computes batch element i end-to-end; no
collectives. Activations are kept "transposed" on device ([channel, seq]) so
per-channel modulation/bias are per-partition scalars; attention scores are
produced directly in [k, q] layout (softmax denominator via a ones-row
appended to V inside the AV matmul). Matmuls run bf16 with fp32 PSUM
accumulation; the residual stream stays fp32.

v2 structural changes vs baseline:
- adaLN mods computed once (was duplicated), in 256-col chunks interleaved
  with the qkv phase; b_ada folded in after the DRAM-transpose readback.
- q AND k inverse-rms folded into qhat/khat via E2-broadcast matmuls
  (removes the kss DRAM roundtrip; exp scale becomes the constant 1/8).
- Scores for a head pair issued back-to-back into one [128,2,512] PSUM tile
  (auto tile_position row packing -> ~2x score throughput), exp over the
  whole [128,1024] in one ACT instruction.
- proj/w3 loops are qch-outer so rms/modulate/w12/output phases pipeline
  per sequence half; attention groups qch-outer so proj fills PE while the
  second half's softmax runs.
- All weight DMAs read host-repacked per-tile-contiguous blocks.
- Engine rebalance: squares/copies/bias-adds on ACT, PSUM->SBUF transposeout
  copies as single strided instructions.
"""

import sys

sys.path.insert(0, "/opt/trn_rl_repo")

import numpy as np
import ml_dtypes

import concourse.bacc as bacc
import concourse.bass as bass
import concourse.mybir as mybir
from concourse.tile import TileContext
from concourse.bass_utils import run_bass_kernel_spmd

F32 = mybir.dt.float32
BF16 = mybir.dt.bfloat16
AF = mybir.ActivationFunctionType
ALU = mybir.AluOpType

B, S, D, H = 8, 1024, 1024, 16
HD = D // H  # 64
INNER = 2730
INNER_P = 2816  # 22*128
P = 128
NT = 8
NKT12 = INNER_P // P  # 22
EPS = 1e-6
NADA = 24  # ada chunks of 256 cols

_CACHE = {}


def _to_pmaj(v):
    return np.ascontiguousarray(v.reshape(-1, P).T)


def _rope_perm():
    ev = np.arange(0, HD, 2)
    od = np.arange(1, HD, 2)
    perm = np.concatenate([ev, od])
    partner = np.concatenate([od, ev])
    return perm, partner


def _pack_rows(w):
    """[K, C] -> per 128-col tile contiguous blocks: out[m*128+p, kt*128+c]
    = w[kt*128+p, m*128+c]; result 2D [n_m*128, K/128*128]."""
    K, C = w.shape
    nk = K // P
    nm = C // P
    out = np.zeros((nm * P, nk * P), np.float32)
    for m in range(nm):
        blk = w[:, m * P : (m + 1) * P]  # [K, 128]
        # [nk, 128p, 128c] -> [128p, nk, 128c]
        out[m * P : (m + 1) * P, :] = (
            blk.reshape(nk, P, P).transpose(1, 0, 2).reshape(P, nk * P)
        )
    return out


def _pack_rows_w(w, colw):
    """Like _pack_rows but with arbitrary col tile width colw."""
    K, C = w.shape
    nk = K // P
    nm = C // colw
    out = np.zeros((nm * P, nk * colw), np.float32)
    for m in range(nm):
        blk = w[:, m * colw : (m + 1) * colw]
        out[m * P : (m + 1) * P, :] = (
            blk.reshape(nk, P, colw).transpose(1, 0, 2).reshape(P, nk * colw)
        )
    return out


def _prep_weights(inp):
    """Host-side layout/dtype prep (reordering/padding only, no math)."""
    perm, partner = _rope_perm()
    chperm = (np.arange(D).reshape(H, HD)[:, perm]).reshape(-1)

    w_qkv, b_qkv = inp["w_qkv"], inp["b_qkv"]
    wq = w_qkv[:, 0:D][:, chperm]
    wk = w_qkv[:, D : 2 * D][:, chperm]
    wv = w_qkv[:, 2 * D :]
    bq = b_qkv[0:D][chperm]
    bk = b_qkv[D : 2 * D][chperm]
    bv = b_qkv[2 * D :]
    wv_ext = np.zeros((D, H * 65), np.float32)
    bv_ext = np.zeros((H * 65,), np.float32)
    for h in range(H):
        wv_ext[:, h * 65 : h * 65 + 64] = wv[:, h * 64 : (h + 1) * 64]
        bv_ext[h * 65 : h * 65 + 64] = bv[h * 64 : (h + 1) * 64]
        bv_ext[h * 65 + 64] = 1.0
    wqk_cat = np.concatenate([wq, wk], axis=1)  # [D, 2048]

    w12, b12 = inp["w12"], inp["b12"]
    w12p = np.zeros((D, 2 * INNER_P), np.float32)
    b12p = np.zeros((2 * INNER_P,), np.float32)
    w12p[:, :INNER] = w12[:, :INNER]
    w12p[:, INNER_P : INNER_P + INNER] = w12[:, INNER:]
    b12p[:INNER] = b12[:INNER]
    b12p[INNER_P : INNER_P + INNER] = b12[INNER:]
    w3p = np.zeros((INNER_P, D), np.float32)
    w3p[:INNER] = inp["w3"]

    # rope tiles [128, S]: two stacked 64-row head-local blocks
    sign = np.where(np.arange(HD) < HD // 2, -1.0, 1.0).astype(np.float32)
    cos, sin = inp["rope_cos"], inp["rope_sin"]

    def rope_tiles(scale_vec):
        c64 = cos[:, perm].T * scale_vec[perm][:, None]
        s64 = (sin[:, perm].T * sign[:, None]) * scale_vec[partner][:, None]
        return (
            np.concatenate([c64, c64], 0).astype(np.float32),
            np.concatenate([s64, s64], 0).astype(np.float32),
        )

    cq, sq = rope_tiles(inp["qn_scale"])
    ck, sk = rope_tiles(inp["kn_scale"])

    E2 = np.zeros((2, P), np.float32)
    E2[0, 0:64] = 1.0
    E2[1, 64:128] = 1.0
    e65 = np.zeros((65, 64), np.float32)
    e65[64, :] = 1.0
    bo2 = np.zeros((P, 2), np.float32)
    bo2[0:64, 0] = 1.0
    bo2[64:128, 1] = 1.0

    bqk_T = np.stack(
        [bq.reshape(NT, P)[m] for m in range(NT)]
        + [bk.reshape(NT, P)[m] for m in range(NT)],
        axis=1,
    )

    return {
        "wqkv_t": _pack_rows(wqk_cat),          # [2048, 1024]
        "wv_t": _pack_rows_w(wv_ext, 260),       # [512, 2080]
        "wproj_t": _pack_rows(inp["w_proj"]),    # [1024, 1024]
        "w12_t": _pack_rows(w12p),               # [5632, 1024]
        "w3_t": _pack_rows(w3p),                 # [1024, 2816]
        "wada_t": _pack_rows_w(inp["w_ada"], 256),  # [3072, 2048]
        "bqk_T": bqk_T, "bv_ext": bv_ext[None, :],
        "b12T": _to_pmaj(b12p), "bprojT": _to_pmaj(inp["b_proj"]),
        "b3T": _to_pmaj(inp["b3"]), "n1T": _to_pmaj(inp["norm1_scale"]),
        "n2T": _to_pmaj(inp["norm2_scale"]), "b_adaT": _to_pmaj(inp["b_ada"]),
        "E2": E2, "e65": e65, "bo2": bo2, "ones1": np.ones((1, P), np.float32),
        "ident": np.eye(P, dtype=np.float32),
        "cos2q": cq, "sin2q": sq, "cos2k": ck, "sin2k": sk,
    }


BF16_NAMES = {
    "wqkv_t", "wv_t", "wproj_t", "w12_t", "w3_t", "wada_t", "bv_ext", "E2",
    "e65", "bo2", "ones1", "cos2q", "sin2q", "cos2k", "sin2k",
}


def build_bass():
    nc = bacc.Bacc("TRN2", target_bir_lowering=False, debug=False, num_devices=8)

    def par(name, shape, dt, out=False):
        return nc.declare_dram_parameter(name, list(shape), dt, isOutput=out)

    d = {
        "x": par("x", [S, D], F32),
        "cT": par("cT", [P, NT], F32),
        "wqkv_t": par("wqkv_t", [2 * D, D], BF16),
        "wv_t": par("wv_t", [4 * P, NT * 260], BF16),
        "wproj_t": par("wproj_t", [D, D], BF16),
        "w12_t": par("w12_t", [2 * INNER_P, D], BF16),
        "w3_t": par("w3_t", [D, INNER_P], BF16),
        "wada_t": par("wada_t", [NADA * P, NT * 256], BF16),
        "bqk_T": par("bqk_T", [P, 16], F32),
        "bv_ext": par("bv_ext", [1, H * 65], BF16),
        "b12T": par("b12T", [P, 2 * NKT12], F32),
        "bprojT": par("bprojT", [P, NT], F32),
        "b3T": par("b3T", [P, NT], F32),
        "n1T": par("n1T", [P, NT], F32),
        "n2T": par("n2T", [P, NT], F32),
        "b_adaT": par("b_adaT", [P, 48], F32),
        "E2": par("E2", [2, P], BF16),
        "e65": par("e65", [65, 64], BF16),
        "bo2": par("bo2", [P, 2], BF16),
        "ones1": par("ones1", [1, P], BF16),
        "ident": par("ident", [P, P], F32),
        "cos2q": par("cos2q", [P, S], BF16),
        "sin2q": par("sin2q", [P, S], BF16),
        "cos2k": par("cos2k", [P, S], BF16),
        "sin2k": par("sin2k", [P, S], BF16),
        "out": par("out", [S, D], F32, out=True),
    }
    mods_dram = nc.dram_tensor("mods_scratch", [1, 6 * D], F32)

    with TileContext(nc) as tc:
        _body(nc, tc, d, mods_dram)
    nc.compile()
    return nc


def _body(nc, tc, d, mods_dram):
    from contextlib import ExitStack

    with ExitStack() as ctx:
        const = ctx.enter_context(tc.tile_pool(name="const", bufs=1))
        persist = ctx.enter_context(tc.tile_pool(name="persist", bufs=1))
        small = ctx.enter_context(tc.tile_pool(name="small", bufs=1))
        scratch = ctx.enter_context(tc.tile_pool(name="scratch", bufs=2))
        # single shared [128,512] PSUM rotation for phases B/C (2 banks)
        mmp = ctx.enter_context(tc.tile_pool(name="mmp", bufs=2, space="PSUM"))

        def load_const(key, shape, dt, pool=None):
            t = (pool or const).tile(list(shape), dt, tag=key, name=key + "_sb")
            nc.sync.dma_start(out=t[:], in_=d[key][:])
            return t

        cT = load_const("cT", [P, NT], F32)
        bqkT = load_const("bqk_T", [P, 16], F32)
        bv = load_const("bv_ext", [1, H * 65], BF16)
        b12T = load_const("b12T", [P, 2 * NKT12], F32)
        bprojT = load_const("bprojT", [P, NT], F32)
        b3T = load_const("b3T", [P, NT], F32)
        n1T = load_const("n1T", [P, NT], F32)
        n2T = load_const("n2T", [P, NT], F32)
        badaT = load_const("b_adaT", [P, 48], F32)
        bo2 = load_const("bo2", [P, 2], BF16)
        e65 = load_const("e65", [65, 64], BF16)
        E2 = load_const("E2", [2, P], BF16)
        ones1 = load_const("ones1", [1, P], BF16)
        ident = load_const("ident", [P, P], F32)
        ones128 = const.tile([P, P], BF16, tag="ones128", name="ones128")
        nc.vector.memset(ones128[:], 1.0)
        eps1 = const.tile([P, 1], F32, tag="eps1", name="eps1")
        nc.vector.memset(eps1[:], EPS)

        # residual stream (fp32) + per-token inverse rms, both persistent
        xT = persist.tile([P, NT, S], F32, tag="xT", name="xT")
        invb = persist.tile([P, S], F32, tag="invb", name="invb")
        h2T = persist.tile([P, NT, S], BF16, tag="h2T", name="h2T")

        def rms_half(half):
            c0 = half * 512
            ms = mmp.tile([P, 512], F32, tag="mm", name="ms")
            for dt in range(NT):
                sq = scratch.tile([P, 512], BF16, tag="sq", name="sq")
                nc.scalar.activation(sq[:], xT[:, dt, c0 : c0 + 512], AF.Square)
                nc.tensor.matmul(
                    ms[:], ones128[:], sq[:], start=(dt == 0), stop=(dt == NT - 1)
                )
            rmsv = scratch.tile([P, 512], F32, tag="rmsv", name="rmsv")
            nc.scalar.activation(rmsv[:], ms[:], AF.Sqrt, bias=eps1[:], scale=1.0 / D)
            nc.vector.reciprocal_approx_fast(invb[:, c0 : c0 + 512], rmsv[:])

        def modulate_half(dstT, aa, sh, half):
            c0 = half * 512
            for dt in range(NT):
                tmp = scratch.tile([P, 512], BF16, tag="modtmp", name="modtmp")
                nc.vector.tensor_mul(
                    tmp[:], xT[:, dt, c0 : c0 + 512], invb[:, c0 : c0 + 512]
                )
                nc.vector.tensor_scalar(
                    dstT[:, dt, c0 : c0 + 512], tmp[:],
                    aa[:, dt : dt + 1], sh[:, dt : dt + 1],
                    op0=ALU.mult, op1=ALU.add,
                )

        # ===== silu(c) (tiny) =====
        cT_silu = small.tile([P, NT], F32, name="cT_silu")
        nc.scalar.activation(cT_silu[:], cT[:], AF.Silu)
        cs_bf = small.tile([P, NT], BF16, name="cs_bf")
        nc.vector.tensor_copy(cs_bf[:], cT_silu[:])

        # ===== attention superblock =====
        with ExitStack() as actx:
            ho = actx.enter_context(tc.tile_pool(name="ho", bufs=1))
            hT = ho.tile([P, NT, S], BF16, tag="hT", name="hT")
            ohat = ho.tile([P, NT, S], BF16, tag="ohat", name="ohat")
            qhat = ho.tile([P, NT, S], BF16, tag="qhat", name="qhat")
            khat = ho.tile([P, NT, S], BF16, tag="khat", name="khat")
            v_sb = ho.tile([P, NT, H * 65], BF16, tag="v", name="v_sb")

            wada_pool = actx.enter_context(tc.tile_pool(name="wada_pool", bufs=2))
            ada_sc = actx.enter_context(tc.tile_pool(name="ada_sc", bufs=2))

            def ada_chunk(n):
                wt = wada_pool.tile([P, NT, 256], BF16, tag="wada", name="wada_t")
                nc.sync.dma_start(
                    out=wt[:],
                    in_=d["wada_t"][n * P : (n + 1) * P, :].rearrange(
                        "p (kt c) -> p kt c", c=256
                    ),
                )
                ps = mmp.tile([P, 512], F32, tag="mm", name="ps_ada")
                for kt in range(NT):
                    nc.tensor.matmul(
                        ps[0:1, 0:256], cs_bf[:, kt : kt + 1], wt[:, kt, :],
                        start=(kt == 0), stop=(kt == NT - 1),
                    )
                mch = ada_sc.tile([1, 256], F32, tag="mch", name="mch")
                nc.scalar.activation(mch[:], ps[0:1, 0:256], AF.Copy)
                nc.sync.dma_start(
                    out=mods_dram[:, n * 256 : (n + 1) * 256], in_=mch[:]
                )

            # mods chunks 0..7 (sh_msa, sc_msa) first — they gate modulate
            for n in range(8):
                ada_chunk(n)

            # ---- Phase B: load x, transpose to [ch, seq] ----
            with tc.tile_pool(name="xin_pool", bufs=3) as xin_pool, tc.tile_pool(
                name="bpsum", bufs=2, space="PSUM"
            ) as bpsum:
                for st in range(NT):
                    xin = xin_pool.tile([P, D], F32, tag="xin", name="xin")
                    nc.sync.dma_start(out=xin[:], in_=d["x"][st * P : (st + 1) * P, :])
                    for g4 in range(2):
                        pt = bpsum.tile([P, 512], F32, tag="bp", name="pt")
                        for j in range(4):
                            dt = g4 * 4 + j
                            nc.tensor.transpose(
                                pt[:, j * P : (j + 1) * P],
                                xin[:, dt * P : (dt + 1) * P],
                                ident[:],
                            )
                        nc.scalar.activation(
                            xT[:, g4 * 4 : g4 * 4 + 4, st * P : (st + 1) * P],
                            pt[:].rearrange("p (j c) -> p j c", c=P), AF.Copy,
                        )
                    if st == 3:
                        rms_half(0)
                    if st == 7:
                        rms_half(1)

            # early mods readback (chunks 0..7 = sh_msa | sc_msa)
            mT16r = small.tile([P, 16], F32, name="mT16r")
            nc.sync.dma_start(
                out=mT16r[:],
                in_=mods_dram.ap()[0, 0:2048].rearrange("(t p) -> p t", p=P),
            )
            mT16 = small.tile([P, 16], F32, name="mT16")
            nc.vector.tensor_add(mT16[:], mT16r[:], badaT[:, 0:16])
            a1 = small.tile([P, NT], F32, name="a1")
            nc.vector.tensor_scalar_add(a1[:], mT16[:, 8:16], 1.0)
            nc.vector.tensor_mul(a1[:], a1[:], n1T[:])
            sh1 = mT16[:, 0:8]
            modulate_half(hT, a1, sh1, 0)
            modulate_half(hT, a1, sh1, 1)

            # ---- Phase C: qkv + rope + norms + v, interleaved with ada ----
            with ExitStack() as cctx:
                ropec = cctx.enter_context(tc.tile_pool(name="ropec", bufs=1))
                qkn = cctx.enter_context(tc.tile_pool(name="qkn", bufs=1))
                wqk_pool = cctx.enter_context(tc.tile_pool(name="wqk_pool", bufs=3))
                wv_pool = cctx.enter_context(tc.tile_pool(name="wv_pool", bufs=2))
                rope_sc = cctx.enter_context(tc.tile_pool(name="rope_sc", bufs=2))
                cpsum = cctx.enter_context(
                    tc.tile_pool(name="cpsum", bufs=2, space="PSUM")
                )

                cos2q = load_const("cos2q", [P, S], BF16, pool=ropec)
                sin2q = load_const("sin2q", [P, S], BF16, pool=ropec)
                cos2k = load_const("cos2k", [P, S], BF16, pool=ropec)
                sin2k = load_const("sin2k", [P, S], BF16, pool=ropec)

                def qkv_mtile(m):
                    isq = m < NT
                    mk = m % NT
                    wt = wqk_pool.tile([P, NT, P], BF16, tag="wqk", name="wqk_t")
                    nc.sync.dma_start(
                        out=wt[:],
                        in_=d["wqkv_t"][m * P : (m + 1) * P, :].rearrange(
                            "p (kt c) -> p kt c", c=P
                        ),
                    )
                    raw = rope_sc.tile([P, S], BF16, tag="raw", name="raw")
                    for sch in range(2):
                        ps = mmp.tile([P, 512], F32, tag="mm", name="ps_qkv")
                        for kt in range(NT):
                            nc.tensor.matmul(
                                ps[:], wt[:, kt, :],
                                hT[:, kt, sch * 512 : (sch + 1) * 512],
                                start=(kt == 0), stop=(kt == NT - 1),
                            )
                        nc.scalar.activation(
                            raw[:, sch * 512 : (sch + 1) * 512], ps[:],
                            AF.Identity, bias=bqkT[:, m : m + 1],
                        )
                    sqs = rope_sc.tile([P, S], BF16, tag="t1", name="sqs")
                    nc.scalar.activation(sqs[:], raw[:], AF.Square)
                    qt = qkn.tile([2, S], F32, tag="qt", name="qt")
                    for sch in range(2):
                        ss = cpsum.tile([P, 512], F32, tag="cp", name="ps_ss")
                        nc.tensor.matmul(
                            ss[0:2, :], bo2[:],
                            sqs[:, sch * 512 : (sch + 1) * 512],
                            start=True, stop=True,
                        )
                        nc.scalar.activation(
                            qt[:, sch * 512 : (sch + 1) * 512], ss[0:2, :],
                            AF.Sqrt, bias=eps1[0:2, :], scale=1.0 / HD,
                        )
                    qr = qkn.tile([2, S], F32, tag="qr", name="qr")
                    nc.vector.reciprocal_approx_fast(qr[:], qt[:])
                    qbf = qkn.tile([2, S], BF16, tag="qbf", name="qbf")
                    nc.vector.tensor_copy(qbf[:], qr[:])
                    rot = rope_sc.tile([P, S], BF16, tag="rot", name="rot")
                    for blk in range(4):
                        b0 = blk * 32
                        srcb = b0 + (32 if blk % 2 == 0 else -32)
                        nc.gpsimd.dma_start(
                            out=rot[b0 : b0 + 32, :], in_=raw[srcb : srcb + 32, :]
                        )
                    t1 = rope_sc.tile([P, S], BF16, tag="t1", name="t1")
                    t2 = rope_sc.tile([P, S], BF16, tag="t2", name="t2")
                    nc.vector.tensor_mul(t1[:], raw[:], (cos2q if isq else cos2k)[:])
                    nc.vector.tensor_mul(t2[:], rot[:], (sin2q if isq else sin2k)[:])
                    t12 = rope_sc.tile([P, S], BF16, tag="rot", name="t12")
                    nc.vector.tensor_add(t12[:], t1[:], t2[:])
                    dst = qhat if isq else khat
                    for sch in range(2):
                        pe = cpsum.tile([P, 512], F32, tag="cp", name="ps_e2")
                        nc.tensor.matmul(
                            pe[:], E2[:], qbf[:, sch * 512 : (sch + 1) * 512],
                            start=True, stop=True,
                        )
                        nc.vector.tensor_mul(
                            dst[:, mk, sch * 512 : (sch + 1) * 512],
                            t12[:, sch * 512 : (sch + 1) * 512], pe[:],
                        )

                def v_chunk(c):
                    wt = wv_pool.tile([P, NT, 260], BF16, tag="wv", name="wv_t")
                    nc.sync.dma_start(
                        out=wt[:],
                        in_=d["wv_t"][c * P : (c + 1) * P, :].rearrange(
                            "p (kt c2) -> p kt c2", c2=260
                        ),
                    )
                    c0 = c * 260
                    for st in range(NT):
                        ps = cpsum.tile([P, 512], F32, tag="cp", name="ps_v")
                        for kt in range(NT):
                            nc.tensor.matmul(
                                ps[:, 0:260], hT[:, kt, st * P : (st + 1) * P],
                                wt[:, kt, :], start=(kt == 0), stop=False,
                            )
                        nc.tensor.matmul(
                            ps[:, 0:260], ones1[:], bv[:, c0 : c0 + 260],
                            start=False, stop=True,
                        )
                        nc.scalar.activation(
                            v_sb[:, st, c0 : c0 + 260], ps[:, 0:260], AF.Copy
                        )

                # interleave: q/k m-tiles, v chunks, remaining ada chunks
                m_order = [0, 8, 1, 9, 2, 10, 3, 11, 4, 12, 5, 13, 6, 14, 7, 15]
                ada_next = 8
                for i, m in enumerate(m_order):
                    qkv_mtile(m)
                    if i in (3, 7, 11, 15):
                        v_chunk(i // 4)
                    if i % 2 == 1 and ada_next < NADA:
                        ada_chunk(ada_next)
                        ada_chunk(ada_next + 1)
                        ada_next += 2

            # full mods readback + derived scalars
            mT48r = small.tile([P, 48], F32, name="mT48r")
            nc.sync.dma_start(
                out=mT48r[:], in_=mods_dram.ap()[0, :].rearrange("(t p) -> p t", p=P)
            )
            mT48 = small.tile([P, 48], F32, name="mT48")
            nc.vector.tensor_add(mT48[:], mT48r[:], badaT[:])
            g1 = mT48[:, 16:24]
            g1b = small.tile([P, NT], F32, name="g1b")
            nc.vector.tensor_mul(g1b[:], g1, bprojT[:])
            a2 = small.tile([P, NT], F32, name="a2")
            nc.vector.tensor_scalar_add(a2[:], mT48[:, 32:40], 1.0)
            nc.vector.tensor_mul(a2[:], a2[:], n2T[:])
            sh2 = mT48[:, 24:32]
            g2 = mT48[:, 40:48]
            g2b3 = small.tile([P, NT], F32, name="g2b3")
            nc.vector.tensor_mul(g2b3[:], g2, b3T[:])

            # ---- Phase D: attention (qch-outer) + proj as PE filler ----
            with ExitStack() as dctx:
                scp = dctx.enter_context(
                    tc.tile_pool(name="scp", bufs=2, space="PSUM")
                )
                avp = dctx.enter_context(
                    tc.tile_pool(name="avp", bufs=1, space="PSUM")
                )
                ptp = dctx.enter_context(tc.tile_pool(name="ptp", bufs=3))
                att_sc = dctx.enter_context(tc.tile_pool(name="att_sc", bufs=2))
                wproj_pool = dctx.enter_context(
                    tc.tile_pool(name="wproj_pool", bufs=3)
                )

                def attn_group(mk, qch):
                    q0 = qch * 512
                    avt = avp.tile([65, 2, 512], F32, tag="av", name="avt")
                    for kt in range(NT):
                        sct = scp.tile([P, 2, 512], F32, tag="sc", name="sct")
                        for hh in range(2):
                            rb = 64 * hh
                            nc.tensor.matmul(
                                sct[:, hh, :],
                                khat[rb : rb + 64, mk, kt * P : (kt + 1) * P],
                                qhat[rb : rb + 64, mk, q0 : q0 + 512],
                                start=True, stop=True,
                            )
                        pt = ptp.tile([P, 2, 512], BF16, tag="pT", name="pt")
                        nc.scalar.activation(pt[:], sct[:], AF.Exp, scale=0.125)
                        for hh in range(2):
                            h = 2 * mk + hh
                            nc.tensor.matmul(
                                avt[:, hh, :], v_sb[:, kt, h * 65 : h * 65 + 65],
                                pt[:, hh, :],
                                start=(kt == 0), stop=(kt == NT - 1),
                            )
                    for hh in range(2):
                        rb = 64 * hh
                        o65b = att_sc.tile([65, 512], BF16, tag="o65b", name="o65b")
                        nc.vector.tensor_copy(o65b[:], avt[:, hh, :])
                        pb = mmp.tile([P, 512], F32, tag="mm", name="ps_bc")
                        nc.tensor.matmul(
                            pb[0:64, :], e65[:], o65b[:], start=True, stop=True
                        )
                        rb64 = att_sc.tile([64, 512], F32, tag="rb64", name="rb64")
                        nc.vector.reciprocal_approx_fast(rb64[:], pb[0:64, :])
                        if hh == 0:
                            nc.vector.tensor_mul(
                                ohat[0:64, mk, q0 : q0 + 512],
                                o65b[0:64, :], rb64[:],
                            )
                        else:
                            ob = att_sc.tile([64, 512], BF16, tag="ob", name="ob")
                            nc.vector.tensor_mul(ob[:], o65b[0:64, :], rb64[:])
                            nc.sync.dma_start(
                                out=ohat[64:128, mk, q0 : q0 + 512], in_=ob[:]
                            )

                def proj_qch(qch):
                    for dt in range(NT):
                        wt = wproj_pool.tile(
                            [P, NT, P], BF16, tag="wproj", name="wproj_t"
                        )
                        nc.sync.dma_start(
                            out=wt[:],
                            in_=d["wproj_t"][dt * P : (dt + 1) * P, :].rearrange(
                                "p (kt c) -> p kt c", c=P
                            ),
                        )
                        ps = mmp.tile([P, 512], F32, tag="mm", name="ps_proj")
                        for kt in range(NT):
                            nc.tensor.matmul(
                                ps[:], wt[:, kt, :],
                                ohat[:, kt, qch * 512 : (qch + 1) * 512],
                                start=(kt == 0), stop=(kt == NT - 1),
                            )
                        nc.vector.affine_then_add(
                            xT[:, dt, qch * 512 : (qch + 1) * 512],
                            ps[:], xT[:, dt, qch * 512 : (qch + 1) * 512],
                            scale=g1[:, dt : dt + 1], bias=g1b[:, dt : dt + 1],
                        )

                for mk in range(NT):
                    attn_group(mk, 0)
                proj_qch(0)
                rms_half(0)
                modulate_half(h2T, a2, sh2, 0)
                for mk in range(NT):
                    attn_group(mk, 1)
                proj_qch(1)
                rms_half(1)
                modulate_half(h2T, a2, sh2, 1)

        # ===== MLP phase =====
        with ExitStack() as mctx:
            mlp = mctx.enter_context(tc.tile_pool(name="mlp", bufs=1))
            mpsum = mctx.enter_context(
                tc.tile_pool(name="mpsum", bufs=6, space="PSUM")
            )

            gg = mlp.tile([P, NKT12, S], BF16, tag="gg", name="gg")
            w3_sb = mlp.tile([P, NT, NKT12 * P], BF16, tag="w3sb", name="w3_sb")
            nc.sync.dma_start(
                out=w3_sb[:],
                in_=d["w3_t"].ap().rearrange("(dt p) c -> p dt c", p=P),
            )

            with tc.tile_pool(name="w12_pool", bufs=3) as w12_pool, tc.tile_pool(
                name="mlp_sc", bufs=2
            ) as mlp_sc:
                for j in range(NKT12):
                    outs = []
                    for part in range(2):
                        m = j + part * NKT12
                        wt = w12_pool.tile([P, NT, P], BF16, tag="w12", name="w12_t")
                        nc.sync.dma_start(
                            out=wt[:],
                            in_=d["w12_t"][m * P : (m + 1) * P, :].rearrange(
                                "p (kt c) -> p kt c", c=P
                            ),
                        )
                        o = mlp_sc.tile([P, S], BF16, tag=f"mlp{part}", name=f"mlp{part}")
                        for sch in range(2):
                            ps = mpsum.tile([P, 512], F32, tag="ps", name="ps_mlp")
                            for kt in range(NT):
                                nc.tensor.matmul(
                                    ps[:], wt[:, kt, :],
                                    h2T[:, kt, sch * 512 : (sch + 1) * 512],
                                    start=(kt == 0), stop=(kt == NT - 1),
                                )
                            nc.scalar.activation(
                                o[:, sch * 512 : (sch + 1) * 512], ps[:],
                                AF.Silu if part == 0 else AF.Identity,
                                bias=b12T[:, m : m + 1],
                            )
                        outs.append(o)
                    nc.vector.tensor_mul(gg[:, j, :], outs[0][:], outs[1][:])

            # w3 + residual 2 (qch-outer) + output transpose per half
            with tc.tile_pool(name="yout", bufs=3) as ypool:
                for qch in range(2):
                    for dt in range(NT):
                        ps = mpsum.tile([P, 512], F32, tag="ps", name="ps_w3")
                        for kt in range(NKT12):
                            nc.tensor.matmul(
                                ps[:], w3_sb[:, dt, kt * P : (kt + 1) * P],
                                gg[:, kt, qch * 512 : (qch + 1) * 512],
                                start=(kt == 0), stop=(kt == NKT12 - 1),
                            )
                        nc.vector.affine_then_add(
                            xT[:, dt, qch * 512 : (qch + 1) * 512],
                            ps[:], xT[:, dt, qch * 512 : (qch + 1) * 512],
                            scale=g2[:, dt : dt + 1], bias=g2b3[:, dt : dt + 1],
                        )
                    for st in range(qch * 4, qch * 4 + 4):
                        y = ypool.tile([P, D], F32, tag="y", name="y")
                        for g4 in range(2):
                            pt = mpsum.tile([P, 512], F32, tag="ps", name="ps_tr2")
                            for j in range(4):
                                dt = g4 * 4 + j
                                nc.tensor.transpose(
                                    pt[:, j * P : (j + 1) * P],
                                    xT[:, dt, st * P : (st + 1) * P],
                                    ident[:],
                                )
                            nc.scalar.activation(
                                y[:, g4 * 512 : (g4 + 1) * 512], pt[:], AF.Copy
                            )
                        nc.sync.dma_start(
                            out=d["out"][st * P : (st + 1) * P, :], in_=y[:]
                        )


def kernel(**inputs):
    inputs = {k: np.asarray(v) for k, v in inputs.items()}
    if "nc" not in _CACHE:
        _CACHE["nc"] = build_bass()
    nc = _CACHE["nc"]

    consts = _prep_weights(inputs)
    base = {}
    for k, v in consts.items():
        if k in BF16_NAMES:
            base[k] = np.ascontiguousarray(v).astype(ml_dtypes.bfloat16)
        else:
            base[k] = np.ascontiguousarray(v).astype(np.float32)

    in_maps = []
    for core in range(B):
        m = dict(base)
        m["x"] = np.ascontiguousarray(inputs["x"][core]).astype(np.float32)
        m["cT"] = _to_pmaj(inputs["c"][core]).astype(np.float32)
        in_maps.append(m)

    res = run_bass_kernel_spmd(
        nc, in_maps, core_ids=list(range(B)), **_CACHE.get("run_kwargs", {})
    )
    _CACHE["last_results"] = res
    return np.stack([res.results[i]["out"] for i in range(B)], axis=0)


if __name__ == "__main__":
    build_bass()
    print("built ok")


# revision 24
# speedup vs baseline: 1.4991x; 1.0222x over previous
"""JiT/DiT transformer block (adaLN + attention + SwiGLU) on 8 TRN2 NeuronCores.

Data-parallel over batch: core i computes batch element i end-to-end; no
collectives. Activations are kept "transposed" on device ([channel, seq]) so
per-channel modulation/bias are per-partition scalars; attention scores are
produced directly in [k, q] layout (softmax denominator via a ones-row
appended to V inside the AV matmul). Matmuls run bf16 with fp32 PSUM
accumulation; the residual stream stays fp32.

v2 structural changes vs baseline:
- adaLN mods computed once (was duplicated), in 256-col chunks interleaved
  with the qkv phase; b_ada folded in after the DRAM-transpose readback.
- q AND k inverse-rms folded into qhat/khat via E2-broadcast matmuls
  (removes the kss DRAM roundtrip; exp scale becomes the constant 1/8).
- Scores for a head pair issued back-to-back into one [128,2,512] PSUM tile
  (auto tile_position row packing -> ~2x score throughput), exp over the
  whole [128,1024] in one ACT instruction.
- proj/w3 loops are qch-outer so rms/modulate/w12/output phases pipeline
  per sequence half; attention groups qch-outer so proj fills PE while the
  second half's softmax runs.
- All weight DMAs read host-repacked per-tile-contiguous blocks.
- Engine rebalance: squares/copies/bias-adds on ACT, PSUM->SBUF transposeout
  copies as single strided instructions.
"""

import sys

sys.path.insert(0, "/opt/trn_rl_repo")

import numpy as np
import ml_dtypes

import concourse.bacc as bacc
import concourse.bass as bass
import concourse.mybir as mybir
from concourse.tile import TileContext
from concourse.bass_utils import run_bass_kernel_spmd

F32 = mybir.dt.float32
BF16 = mybir.dt.bfloat16
FP8 = mybir.dt.float8e4
U32 = mybir.dt.uint32
DR = mybir.MatmulPerfMode.DoubleRow
AF = mybir.ActivationFunctionType
ALU = mybir.AluOpType

B, S, D, H = 8, 1024, 1024, 16
HD = D // H  # 64
INNER = 2730
INNER_P = 2816  # 22*128
P = 128
NT = 8
NKT12 = INNER_P // P  # 22
EPS = 1e-6
NADA = 24  # ada chunks of 256 cols

_CACHE = {}


def _to_pmaj(v):
    return np.ascontiguousarray(v.reshape(-1, P).T)


def _rope_perm():
    ev = np.arange(0, HD, 2)
    od = np.arange(1, HD, 2)
    perm = np.concatenate([ev, od])
    partner = np.concatenate([od, ev])
    return perm, partner


def _pack_rows(w):
    """[K, C] -> per 128-col tile contiguous blocks: out[m*128+p, kt*128+c]
    = w[kt*128+p, m*128+c]; result 2D [n_m*128, K/128*128]."""
    K, C = w.shape
    nk = K // P
    nm = C // P
    out = np.zeros((nm * P, nk * P), np.float32)
    for m in range(nm):
        blk = w[:, m * P : (m + 1) * P]  # [K, 128]
        # [nk, 128p, 128c] -> [128p, nk, 128c]
        out[m * P : (m + 1) * P, :] = (
            blk.reshape(nk, P, P).transpose(1, 0, 2).reshape(P, nk * P)
        )
    return out


def _pack_rows_w(w, colw):
    """Like _pack_rows but with arbitrary col tile width colw."""
    K, C = w.shape
    nk = K // P
    nm = C // colw
    out = np.zeros((nm * P, nk * colw), np.float32)
    for m in range(nm):
        blk = w[:, m * colw : (m + 1) * colw]
        out[m * P : (m + 1) * P, :] = (
            blk.reshape(nk, P, colw).transpose(1, 0, 2).reshape(P, nk * colw)
        )
    return out


def _prep_weights(inp):
    """Host-side layout/dtype prep (reordering/padding only, no math)."""
    perm, partner = _rope_perm()
    chperm = (np.arange(D).reshape(H, HD)[:, perm]).reshape(-1)

    w_qkv, b_qkv = inp["w_qkv"], inp["b_qkv"]
    wq = w_qkv[:, 0:D][:, chperm]
    wk = w_qkv[:, D : 2 * D][:, chperm]
    wv = w_qkv[:, 2 * D :]
    bq = b_qkv[0:D][chperm]
    bk = b_qkv[D : 2 * D][chperm]
    bv = b_qkv[2 * D :]
    wv_ext = np.zeros((D, H * 65), np.float32)
    bv_ext = np.zeros((H * 65,), np.float32)
    for h in range(H):
        wv_ext[:, h * 65 : h * 65 + 64] = wv[:, h * 64 : (h + 1) * 64]
        bv_ext[h * 65 : h * 65 + 64] = bv[h * 64 : (h + 1) * 64]
        bv_ext[h * 65 + 64] = 1.0
    wqk_cat = np.concatenate([wq, wk], axis=1)  # [D, 2048]

    w12, b12 = inp["w12"], inp["b12"]
    w12p = np.zeros((D, 2 * INNER_P), np.float32)
    b12p = np.zeros((2 * INNER_P,), np.float32)
    w12p[:, :INNER] = w12[:, :INNER]
    w12p[:, INNER_P : INNER_P + INNER] = w12[:, INNER:]
    b12p[:INNER] = b12[:INNER]
    b12p[INNER_P : INNER_P + INNER] = b12[INNER:]
    w3p = np.zeros((INNER_P, D), np.float32)
    w3p[:INNER] = inp["w3"]
    # fp8 weights are stored pre-scaled by 64 (most mass would be denormal
    # in e4m3 otherwise); compensated by scale=1/64 on the consumer side
    w12p *= 64.0
    w3p *= 64.0

    # rope tiles [128, S]: two stacked 64-row head-local blocks
    sign = np.where(np.arange(HD) < HD // 2, -1.0, 1.0).astype(np.float32)
    cos, sin = inp["rope_cos"], inp["rope_sin"]

    def rope_tiles(scale_vec):
        c64 = cos[:, perm].T * scale_vec[perm][:, None]
        s64 = (sin[:, perm].T * sign[:, None]) * scale_vec[partner][:, None]
        return (
            np.concatenate([c64, c64], 0).astype(np.float32),
            np.concatenate([s64, s64], 0).astype(np.float32),
        )

    cq, sq = rope_tiles(inp["qn_scale"])
    ck, sk = rope_tiles(inp["kn_scale"])

    E2 = np.zeros((2, P), np.float32)
    E2[0, 0:64] = 1.0
    E2[1, 64:128] = 1.0
    e65 = np.zeros((65, 64), np.float32)
    e65[64, :] = 1.0
    bo2 = np.zeros((P, 2), np.float32)
    bo2[0:64, 0] = 1.0
    bo2[64:128, 1] = 1.0

    bqk_T = np.stack(
        [bq.reshape(NT, P)[m] for m in range(NT)]
        + [bk.reshape(NT, P)[m] for m in range(NT)],
        axis=1,
    )

    return {
        "wqkv_t": _pack_rows(wqk_cat),          # [2048, 1024]
        "wv_t": _pack_rows_w(wv_ext, 260),       # [512, 2080]
        "wproj_t": _pack_rows(inp["w_proj"]),    # [1024, 1024]
        "w12_t": _pack_rows(w12p),               # [5632, 1024]
        "w3_t": _pack_rows(w3p),                 # [1024, 2816]
        "wada_t": _pack_rows_w(inp["w_ada"], 256),  # [3072, 2048]
        "bqk_T": bqk_T, "bv_ext": bv_ext[None, :],
        "b12T": _to_pmaj(b12p), "bprojT": _to_pmaj(inp["b_proj"]),
        "b3T": _to_pmaj(inp["b3"]), "n1T": _to_pmaj(inp["norm1_scale"]),
        "n2T": _to_pmaj(inp["norm2_scale"]), "b_adaT": _to_pmaj(inp["b_ada"]),
        "E2": E2, "e65": e65, "bo2": bo2, "ones1": np.ones((1, P), np.float32),
        "ident": np.eye(P, dtype=np.float32),
        "cos2q": cq, "sin2q": sq, "cos2k": ck, "sin2k": sk,
    }


BF16_NAMES = {
    "wqkv_t", "wv_t", "wproj_t", "wada_t", "bv_ext", "E2",
    "e65", "bo2", "ones1", "cos2q", "sin2q", "cos2k", "sin2k",
}
FP8_NAMES = {"w12_t", "w3_t"}

# rsqrt-via-exp bit trick constants: for fp32 v with integer bits I,
# log2(v) ~ I/2^23 - (127 - mu); rsqrt(ss/64) = exp(scale*I + bias)
LN2 = float(np.log(2.0))
RSQ_MU = 0.0430
RSQ_SCALE = -LN2 / (2.0**24)
RSQ_BIAS = 0.5 * LN2 * (127.0 + 6.0 - RSQ_MU)


def build_bass():
    nc = bacc.Bacc("TRN2", target_bir_lowering=False, debug=False, num_devices=8)

    def par(name, shape, dt, out=False):
        return nc.declare_dram_parameter(name, list(shape), dt, isOutput=out)

    d = {
        "x": par("x", [S, D], F32),
        "cT": par("cT", [P, NT], F32),
        "wqkv_t": par("wqkv_t", [2 * D, D], BF16),
        "wv_t": par("wv_t", [4 * P, NT * 260], BF16),
        "wproj_t": par("wproj_t", [D, D], BF16),
        "w12_t": par("w12_t", [2 * INNER_P, D], FP8),
        "w3_t": par("w3_t", [D, INNER_P], FP8),
        "wada_t": par("wada_t", [NADA * P, NT * 256], BF16),
        "bqk_T": par("bqk_T", [P, 16], F32),
        "bv_ext": par("bv_ext", [1, H * 65], BF16),
        "b12T": par("b12T", [P, 2 * NKT12], F32),
        "bprojT": par("bprojT", [P, NT], F32),
        "b3T": par("b3T", [P, NT], F32),
        "n1T": par("n1T", [P, NT], F32),
        "n2T": par("n2T", [P, NT], F32),
        "b_adaT": par("b_adaT", [P, 48], F32),
        "E2": par("E2", [2, P], BF16),
        "e65": par("e65", [65, 64], BF16),
        "bo2": par("bo2", [P, 2], BF16),
        "ones1": par("ones1", [1, P], BF16),
        "ident": par("ident", [P, P], F32),
        "cos2q": par("cos2q", [P, S], BF16),
        "sin2q": par("sin2q", [P, S], BF16),
        "cos2k": par("cos2k", [P, S], BF16),
        "sin2k": par("sin2k", [P, S], BF16),
        "out": par("out", [S, D], F32, out=True),
    }
    mods_dram = nc.dram_tensor("mods_scratch", [1, 6 * D], F32)

    with TileContext(nc) as tc:
        _body(nc, tc, d, mods_dram)
    nc.compile()
    return nc


def _body(nc, tc, d, mods_dram):
    from contextlib import ExitStack

    with ExitStack() as ctx:
        const = ctx.enter_context(tc.tile_pool(name="const", bufs=1))
        persist = ctx.enter_context(tc.tile_pool(name="persist", bufs=1))
        small = ctx.enter_context(tc.tile_pool(name="small", bufs=1))
        scratch = ctx.enter_context(tc.tile_pool(name="scratch", bufs=2))
        # single shared [128,512] PSUM rotation for phases B/C (2 banks)
        mmp = ctx.enter_context(tc.tile_pool(name="mmp", bufs=2, space="PSUM"))

        def load_const(key, shape, dt, pool=None):
            t = (pool or const).tile(list(shape), dt, tag=key, name=key + "_sb")
            nc.sync.dma_start(out=t[:], in_=d[key][:])
            return t

        cT = load_const("cT", [P, NT], F32)
        bqkT = load_const("bqk_T", [P, 16], F32)
        bv = load_const("bv_ext", [1, H * 65], BF16)
        b12T = load_const("b12T", [P, 2 * NKT12], F32)
        bprojT = load_const("bprojT", [P, NT], F32)
        b3T = load_const("b3T", [P, NT], F32)
        n1T = load_const("n1T", [P, NT], F32)
        n2T = load_const("n2T", [P, NT], F32)
        badaT = load_const("b_adaT", [P, 48], F32)
        bo2 = load_const("bo2", [P, 2], BF16)
        e65 = load_const("e65", [65, 64], BF16)
        E2 = load_const("E2", [2, P], BF16)
        ones1 = load_const("ones1", [1, P], BF16)
        ident = load_const("ident", [P, P], F32)
        ones128 = const.tile([P, P], BF16, tag="ones128", name="ones128")
        nc.vector.memset(ones128[:], 1.0)
        eps1 = const.tile([P, 1], F32, tag="eps1", name="eps1")
        nc.vector.memset(eps1[:], EPS)
        rsqb = const.tile([P, 1], F32, tag="rsqb", name="rsqb")
        nc.vector.memset(rsqb[:], RSQ_BIAS)

        # residual stream (fp32) + per-token inverse rms, both persistent
        xT = persist.tile([P, NT, S], F32, tag="xT", name="xT")
        invb = persist.tile([P, S], F32, tag="invb", name="invb")
        h2T = persist.tile([P, NT, S], FP8, tag="h2T", name="h2T")

        def rms_half(half):
            c0 = half * 512
            ms = mmp.tile([P, 512], F32, tag="mm", name="ms")
            for dt in range(NT):
                sq = scratch.tile([P, 512], BF16, tag="sq", name="sq")
                nc.scalar.activation(sq[:], xT[:, dt, c0 : c0 + 512], AF.Square)
                nc.tensor.matmul(
                    ms[:], ones128[:], sq[:], start=(dt == 0), stop=(dt == NT - 1)
                )
            rmsv = scratch.tile([P, 512], F32, tag="rmsv", name="rmsv")
            nc.scalar.activation(rmsv[:], ms[:], AF.Sqrt, bias=eps1[:], scale=1.0 / D)
            nc.vector.reciprocal_approx_fast(invb[:, c0 : c0 + 512], rmsv[:])

        def modulate_half(dstT, aa, sh, half):
            c0 = half * 512
            for dt in range(NT):
                tmp = scratch.tile([P, 512], BF16, tag="modtmp", name="modtmp")
                nc.vector.tensor_mul(
                    tmp[:], xT[:, dt, c0 : c0 + 512], invb[:, c0 : c0 + 512]
                )
                nc.vector.tensor_scalar(
                    dstT[:, dt, c0 : c0 + 512], tmp[:],
                    aa[:, dt : dt + 1], sh[:, dt : dt + 1],
                    op0=ALU.mult, op1=ALU.add,
                )

        # ===== silu(c) (tiny) =====
        cT_silu = small.tile([P, NT], F32, name="cT_silu")
        nc.scalar.activation(cT_silu[:], cT[:], AF.Silu)
        cs_bf = small.tile([P, NT], BF16, name="cs_bf")
        nc.vector.tensor_copy(cs_bf[:], cT_silu[:])

        # ===== attention superblock =====
        with ExitStack() as actx:
            ho = actx.enter_context(tc.tile_pool(name="ho", bufs=1))
            hT = ho.tile([P, NT, S], BF16, tag="hT", name="hT")
            ohat = ho.tile([P, NT, S], BF16, tag="ohat", name="ohat")
            qhat = ho.tile([P, NT, S], BF16, tag="qhat", name="qhat")
            khat = ho.tile([P, NT, S], BF16, tag="khat", name="khat")
            v_sb = ho.tile([P, NT, H * 65], BF16, tag="v", name="v_sb")

            wada_pool = actx.enter_context(tc.tile_pool(name="wada_pool", bufs=2))
            ada_sc = actx.enter_context(tc.tile_pool(name="ada_sc", bufs=2))

            def ada_chunk(n):
                wt = wada_pool.tile([P, NT, 256], BF16, tag="wada", name="wada_t")
                nc.sync.dma_start(
                    out=wt[:],
                    in_=d["wada_t"][n * P : (n + 1) * P, :].rearrange(
                        "p (kt c) -> p kt c", c=256
                    ),
                )
                ps = mmp.tile([P, 512], F32, tag="mm", name="ps_ada")
                for kt in range(NT):
                    nc.tensor.matmul(
                        ps[0:1, 0:256], cs_bf[:, kt : kt + 1], wt[:, kt, :],
                        start=(kt == 0), stop=(kt == NT - 1),
                    )
                mch = ada_sc.tile([1, 256], F32, tag="mch", name="mch")
                nc.scalar.activation(mch[:], ps[0:1, 0:256], AF.Copy)
                nc.sync.dma_start(
                    out=mods_dram[:, n * 256 : (n + 1) * 256], in_=mch[:]
                )

            # mods chunks 0..7 (sh_msa, sc_msa) first — they gate modulate
            for n in range(8):
                ada_chunk(n)

            # ---- Phase B: load x, transpose to [ch, seq] ----
            with tc.tile_pool(name="xin_pool", bufs=3) as xin_pool, tc.tile_pool(
                name="bpsum", bufs=2, space="PSUM"
            ) as bpsum:
                for st in range(NT):
                    xin = xin_pool.tile([P, D], F32, tag="xin", name="xin")
                    nc.sync.dma_start(out=xin[:], in_=d["x"][st * P : (st + 1) * P, :])
                    for g4 in range(2):
                        pt = bpsum.tile([P, 512], F32, tag="bp", name="pt")
                        for j in range(4):
                            dt = g4 * 4 + j
                            nc.tensor.transpose(
                                pt[:, j * P : (j + 1) * P],
                                xin[:, dt * P : (dt + 1) * P],
                                ident[:],
                            )
                        nc.scalar.activation(
                            xT[:, g4 * 4 : g4 * 4 + 4, st * P : (st + 1) * P],
                            pt[:].rearrange("p (j c) -> p j c", c=P), AF.Copy,
                        )
                    if st == 3:
                        rms_half(0)
                    if st == 7:
                        rms_half(1)

            # early mods readback (chunks 0..7 = sh_msa | sc_msa)
            mT16r = small.tile([P, 16], F32, name="mT16r")
            nc.sync.dma_start(
                out=mT16r[:],
                in_=mods_dram.ap()[0, 0:2048].rearrange("(t p) -> p t", p=P),
            )
            mT16 = small.tile([P, 16], F32, name="mT16")
            nc.vector.tensor_add(mT16[:], mT16r[:], badaT[:, 0:16])
            a1 = small.tile([P, NT], F32, name="a1")
            nc.vector.tensor_scalar_add(a1[:], mT16[:, 8:16], 1.0)
            nc.vector.tensor_mul(a1[:], a1[:], n1T[:])
            sh1 = mT16[:, 0:8]
            modulate_half(hT, a1, sh1, 0)
            modulate_half(hT, a1, sh1, 1)

            # ---- Phase C+D: qkv/rope/v interleaved with attention ----
            with ExitStack() as cctx:
                ropec = cctx.enter_context(tc.tile_pool(name="ropec", bufs=1))
                qkn = cctx.enter_context(tc.tile_pool(name="qkn", bufs=1))
                wqk_pool = cctx.enter_context(tc.tile_pool(name="wqk_pool", bufs=3))
                wv_pool = cctx.enter_context(tc.tile_pool(name="wv_pool", bufs=2))
                rope_sc = cctx.enter_context(tc.tile_pool(name="rope_sc", bufs=2))
                cpsum = cctx.enter_context(
                    tc.tile_pool(name="cpsum", bufs=2, space="PSUM")
                )
                scp = cctx.enter_context(
                    tc.tile_pool(name="scp", bufs=2, space="PSUM")
                )
                avp = cctx.enter_context(
                    tc.tile_pool(name="avp", bufs=1, space="PSUM")
                )
                ptp = cctx.enter_context(tc.tile_pool(name="ptp", bufs=3))
                att_sc = cctx.enter_context(tc.tile_pool(name="att_sc", bufs=1))
                wproj_pool = cctx.enter_context(
                    tc.tile_pool(name="wproj_pool", bufs=2)
                )

                cos2q = load_const("cos2q", [P, S], BF16, pool=ropec)
                sin2q = load_const("sin2q", [P, S], BF16, pool=ropec)
                cos2k = load_const("cos2k", [P, S], BF16, pool=ropec)
                sin2k = load_const("sin2k", [P, S], BF16, pool=ropec)

                def qkv_mtile(m):
                    isq = m < NT
                    mk = m % NT
                    wt = wqk_pool.tile([P, NT, P], BF16, tag="wqk", name="wqk_t")
                    nc.sync.dma_start(
                        out=wt[:],
                        in_=d["wqkv_t"][m * P : (m + 1) * P, :].rearrange(
                            "p (kt c) -> p kt c", c=P
                        ),
                    )
                    raw = rope_sc.tile([P, S], BF16, tag="raw", name="raw")
                    for sch in range(2):
                        ps = mmp.tile([P, 512], F32, tag="mm", name="ps_qkv")
                        for kt in range(NT):
                            nc.tensor.matmul(
                                ps[:], wt[:, kt, :],
                                hT[:, kt, sch * 512 : (sch + 1) * 512],
                                start=(kt == 0), stop=(kt == NT - 1),
                            )
                        nc.scalar.activation(
                            raw[:, sch * 512 : (sch + 1) * 512], ps[:],
                            AF.Identity, bias=bqkT[:, m : m + 1],
                        )
                    sqs = rope_sc.tile([P, S], BF16, tag="t1", name="sqs")
                    nc.vector.tensor_mul(sqs[:], raw[:], raw[:])
                    # rsqrt(ss/64) via exponent-bit log approx folded into Exp
                    # (keeps the whole C+D window inside the exp table set)
                    qf = qkn.tile([2, S], F32, tag="qf", name="qf")
                    qbf = qkn.tile([2, S], BF16, tag="qbf", name="qbf")
                    for sch in range(2):
                        ss = cpsum.tile([P, 512], F32, tag="cp", name="ps_ss")
                        nc.tensor.matmul(
                            ss[0:2, :], bo2[:],
                            sqs[:, sch * 512 : (sch + 1) * 512],
                            start=True, stop=True,
                        )
                        nc.vector.tensor_copy(
                            qf[:, sch * 512 : (sch + 1) * 512],
                            ss[0:2, :].bitcast(U32),
                        )
                        nc.scalar.activation(
                            qbf[:, sch * 512 : (sch + 1) * 512],
                            qf[:, sch * 512 : (sch + 1) * 512],
                            AF.Exp, bias=rsqb[0:2, :], scale=RSQ_SCALE,
                        )
                    rot = rope_sc.tile([P, S], BF16, tag="rot", name="rot")
                    for blk in range(4):
                        b0 = blk * 32
                        srcb = b0 + (32 if blk % 2 == 0 else -32)
                        nc.gpsimd.dma_start(
                            out=rot[b0 : b0 + 32, :], in_=raw[srcb : srcb + 32, :]
                        )
                    t1 = rope_sc.tile([P, S], BF16, tag="t1", name="t1")
                    t2 = rope_sc.tile([P, S], BF16, tag="t2", name="t2")
                    nc.vector.tensor_mul(t1[:], raw[:], (cos2q if isq else cos2k)[:])
                    nc.vector.tensor_mul(t2[:], rot[:], (sin2q if isq else sin2k)[:])
                    t12 = rope_sc.tile([P, S], BF16, tag="rot", name="t12")
                    nc.vector.tensor_add(t12[:], t1[:], t2[:])
                    dst = qhat if isq else khat
                    for sch in range(2):
                        pe = cpsum.tile([P, 512], F32, tag="cp", name="ps_e2")
                        nc.tensor.matmul(
                            pe[:], E2[:], qbf[:, sch * 512 : (sch + 1) * 512],
                            start=True, stop=True,
                        )
                        nc.vector.tensor_mul(
                            dst[:, mk, sch * 512 : (sch + 1) * 512],
                            t12[:, sch * 512 : (sch + 1) * 512], pe[:],
                        )

                def v_chunk(c):
                    wt = wv_pool.tile([P, NT, 260], BF16, tag="wv", name="wv_t")
                    nc.sync.dma_start(
                        out=wt[:],
                        in_=d["wv_t"][c * P : (c + 1) * P, :].rearrange(
                            "p (kt c2) -> p kt c2", c2=260
                        ),
                    )
                    c0 = c * 260
                    for st in range(NT):
                        ps = cpsum.tile([P, 512], F32, tag="cp", name="ps_v")
                        for kt in range(NT):
                            nc.tensor.matmul(
                                ps[:, 0:260], hT[:, kt, st * P : (st + 1) * P],
                                wt[:, kt, :], start=(kt == 0), stop=False,
                            )
                        nc.tensor.matmul(
                            ps[:, 0:260], ones1[:], bv[:, c0 : c0 + 260],
                            start=False, stop=True,
                        )
                        nc.vector.tensor_copy(
                            v_sb[:, st, c0 : c0 + 260], ps[:, 0:260]
                        )

                def attn_group(mk, qch):
                    q0 = qch * 512
                    avt = avp.tile([65, 2, 512], F32, tag="av", name="avt")
                    for kt in range(NT):
                        pts = []
                        for hh in range(2):
                            rb = 64 * hh
                            sct = scp.tile([P, 512], F32, tag="sc", name="sct")
                            nc.tensor.matmul(
                                sct[:],
                                khat[rb : rb + 64, mk, kt * P : (kt + 1) * P],
                                qhat[rb : rb + 64, mk, q0 : q0 + 512],
                                start=True, stop=True,
                            )
                            pt = ptp.tile([P, 512], BF16, tag="pT", name="pt")
                            nc.scalar.activation(pt[:], sct[:], AF.Exp, scale=0.125)
                            pts.append(pt)
                        for hh in range(2):
                            h = 2 * mk + hh
                            nc.tensor.matmul(
                                avt[:, hh, :], v_sb[:, kt, h * 65 : h * 65 + 65],
                                pts[hh][:],
                                start=(kt == 0), stop=(kt == NT - 1),
                            )
                    for hh in range(2):
                        o65b = att_sc.tile([65, 512], BF16, tag="o65b", name="o65b")
                        nc.vector.tensor_copy(o65b[:], avt[:, hh, :])
                        pb = scp.tile([P, 512], F32, tag="sc", name="ps_bc")
                        nc.tensor.matmul(
                            pb[0:64, :], e65[:], o65b[:], start=True, stop=True
                        )
                        rb64 = att_sc.tile([64, 512], F32, tag="rb64", name="rb64")
                        nc.vector.reciprocal_approx_fast(rb64[:], pb[0:64, :])
                        if hh == 0:
                            nc.vector.tensor_mul(
                                ohat[0:64, mk, q0 : q0 + 512],
                                o65b[0:64, :], rb64[:],
                            )
                        else:
                            ob = att_sc.tile([64, 512], BF16, tag="ob", name="ob")
                            nc.vector.tensor_mul(ob[:], o65b[0:64, :], rb64[:])
                            nc.sync.dma_start(
                                out=ohat[64:128, mk, q0 : q0 + 512], in_=ob[:]
                            )

                def proj_qch(qch):
                    for dt in range(NT):
                        wt = wproj_pool.tile(
                            [P, NT, P], BF16, tag="wproj", name="wproj_t"
                        )
                        nc.sync.dma_start(
                            out=wt[:],
                            in_=d["wproj_t"][dt * P : (dt + 1) * P, :].rearrange(
                                "p (kt c) -> p kt c", c=P
                            ),
                        )
                        ps = mmp.tile([P, 512], F32, tag="mm", name="ps_proj")
                        for kt in range(NT):
                            nc.tensor.matmul(
                                ps[:], wt[:, kt, :],
                                ohat[:, kt, qch * 512 : (qch + 1) * 512],
                                start=(kt == 0), stop=(kt == NT - 1),
                            )
                        nc.vector.affine_then_add(
                            xT[:, dt, qch * 512 : (qch + 1) * 512],
                            ps[:], xT[:, dt, qch * 512 : (qch + 1) * 512],
                            scale=g1[:, dt : dt + 1], bias=g1b[:, dt : dt + 1],
                        )

                # interleave: q/k m-tiles, v chunks, ada chunks, attention
                m_order = [0, 8, 1, 9, 2, 10, 3, 11, 4, 12, 5, 13, 6, 14, 7, 15]
                ada_next = 8
                for i, m in enumerate(m_order):
                    qkv_mtile(m)
                    if i in (3, 7, 11, 15):
                        v_chunk(i // 4)
                    if i % 2 == 1 and ada_next < NADA:
                        ada_chunk(ada_next)
                        ada_chunk(ada_next + 1)
                        ada_next += 2
                    if i >= 3 and i % 2 == 1:
                        attn_group((i - 3) // 2, 0)
                for mk in range(1 + NT - 2, NT):
                    attn_group(mk, 0)

                # full mods readback + derived scalars
                mT48r = small.tile([P, 48], F32, name="mT48r")
                nc.sync.dma_start(
                    out=mT48r[:],
                    in_=mods_dram.ap()[0, :].rearrange("(t p) -> p t", p=P),
                )
                mT48 = small.tile([P, 48], F32, name="mT48")
                nc.vector.tensor_add(mT48[:], mT48r[:], badaT[:])
                g1 = mT48[:, 16:24]
                g1b = small.tile([P, NT], F32, name="g1b")
                nc.vector.tensor_mul(g1b[:], g1, bprojT[:])
                a2 = small.tile([P, NT], F32, name="a2")
                nc.vector.tensor_scalar_add(a2[:], mT48[:, 32:40], 1.0)
                nc.vector.tensor_mul(a2[:], a2[:], n2T[:])
                sh2 = mT48[:, 24:32]
                g2 = mT48[:, 40:48]
                g2b3 = small.tile([P, NT], F32, name="g2b3")
                nc.vector.tensor_mul(g2b3[:], g2, b3T[:])
                g2s = small.tile([P, NT], F32, name="g2s")
                nc.scalar.mul(g2s[:], g2, 1.0 / 64.0)

                # second attention wave (qch=1); proj fills PE gaps
                for mk in range(NT):
                    attn_group(mk, 1)
                proj_qch(0)
                proj_qch(1)
                # rms2/mod2 after all exps (sqrt lives in another table set)
                rms_half(0)
                modulate_half(h2T, a2, sh2, 0)
                rms_half(1)
                modulate_half(h2T, a2, sh2, 1)

        # ===== MLP phase =====
        with ExitStack() as mctx:
            mlp = mctx.enter_context(tc.tile_pool(name="mlp", bufs=1))
            mpsum = mctx.enter_context(
                tc.tile_pool(name="mpsum", bufs=6, space="PSUM")
            )

            gg = mlp.tile([P, NKT12, S], FP8, tag="gg", name="gg")
            w3_sb = mlp.tile([P, NT, NKT12 * P], FP8, tag="w3sb", name="w3_sb")
            nc.sync.dma_start(
                out=w3_sb[:],
                in_=d["w3_t"][:, :].rearrange("(dt p) c -> p dt c", p=P),
            )

            with tc.tile_pool(name="w12_pool", bufs=3) as w12_pool, tc.tile_pool(
                name="mlp_sc", bufs=2
            ) as mlp_sc:
                for j in range(NKT12):
                    outs = []
                    for part in range(2):
                        m = j + part * NKT12
                        wt = w12_pool.tile([P, NT, P], FP8, tag="w12", name="w12_t")
                        nc.sync.dma_start(
                            out=wt[:],
                            in_=d["w12_t"][m * P : (m + 1) * P, :].rearrange(
                                "p (kt c) -> p kt c", c=P
                            ),
                        )
                        o = mlp_sc.tile([P, S], BF16, tag=f"mlp{part}", name=f"mlp{part}")
                        for sch in range(2):
                            ps = mpsum.tile([P, 512], F32, tag="ps", name="ps_mlp")
                            for ktp in range(NT // 2):
                                nc.tensor.matmul(
                                    ps[:], wt[:, 2 * ktp : 2 * ktp + 2, :],
                                    h2T[
                                        :, 2 * ktp : 2 * ktp + 2,
                                        sch * 512 : (sch + 1) * 512,
                                    ],
                                    start=(ktp == 0), stop=(ktp == NT // 2 - 1),
                                    perf_mode=DR,
                                )
                            nc.scalar.activation(
                                o[:, sch * 512 : (sch + 1) * 512], ps[:],
                                AF.Silu if part == 0 else AF.Identity,
                                bias=b12T[:, m : m + 1], scale=1.0 / 64.0,
                            )
                        outs.append(o)
                    nc.vector.tensor_mul(gg[:, j, :], outs[0][:], outs[1][:])

            # w3 + residual 2 (qch-outer) + output transpose per half
            with tc.tile_pool(name="yout", bufs=3) as ypool:
                for qch in range(2):
                    for dt in range(NT):
                        ps = mpsum.tile([P, 512], F32, tag="ps", name="ps_w3")
                        for ktp in range(NKT12 // 2):
                            nc.tensor.matmul(
                                ps[:],
                                w3_sb[
                                    :, dt, 2 * ktp * P : (2 * ktp + 2) * P
                                ].rearrange("p (g c) -> p g c", g=2),
                                gg[
                                    :, 2 * ktp : 2 * ktp + 2,
                                    qch * 512 : (qch + 1) * 512,
                                ],
                                start=(ktp == 0), stop=(ktp == NKT12 // 2 - 1),
                                perf_mode=DR,
                            )
                        nc.vector.affine_then_add(
                            xT[:, dt, qch * 512 : (qch + 1) * 512],
                            ps[:], xT[:, dt, qch * 512 : (qch + 1) * 512],
                            scale=g2s[:, dt : dt + 1], bias=g2b3[:, dt : dt + 1],
                        )
                    for st in range(qch * 4, qch * 4 + 4):
                        y = ypool.tile([P, D], F32, tag="y", name="y")
                        for g4 in range(2):
                            pt = mpsum.tile([P, 512], F32, tag="ps", name="ps_tr2")
                            for j in range(4):
                                dt = g4 * 4 + j
                                nc.tensor.transpose(
                                    pt[:, j * P : (j + 1) * P],
                                    xT[:, dt, st * P : (st + 1) * P],
                                    ident[:],
                                )
                            nc.scalar.activation(
                                y[:, g4 * 512 : (g4 + 1) * 512], pt[:], AF.Copy
                            )
                        nc.sync.dma_start(
                            out=d["out"][st * P : (st + 1) * P, :], in_=y[:]
                        )


def kernel(**inputs):
    inputs = {k: np.asarray(v) for k, v in inputs.items()}
    if "nc" not in _CACHE:
        _CACHE["nc"] = build_bass()
    nc = _CACHE["nc"]

    consts = _prep_weights(inputs)
    base = {}
    for k, v in consts.items():
        if k in BF16_NAMES:
            base[k] = np.ascontiguousarray(v).astype(ml_dtypes.bfloat16)
        elif k in FP8_NAMES:
            base[k] = np.ascontiguousarray(v).astype(ml_dtypes.float8_e4m3fn)
        else:
            base[k] = np.ascontiguousarray(v).astype(np.float32)

    in_maps = []
    for core in range(B):
        m = dict(base)
        m["x"] = np.ascontiguousarray(inputs["x"][core]).astype(np.float32)
        m["cT"] = _to_pmaj(inputs["c"][core]).astype(np.float32)
        in_maps.append(m)

    res = run_bass_kernel_spmd(
        nc, in_maps, core_ids=list(range(B)), **_CACHE.get("run_kwargs", {})
    )
    _CACHE["last_results"] = res
    return np.stack([res.results[i]["out"] for i in range(B)], axis=0)


if __name__ == "__main__":
    build_bass()
    print("built ok")


# revision 31
# speedup vs baseline: 1.6589x; 1.1067x over previous
"""JiT/DiT transformer block (adaLN + attention + SwiGLU) on 8 TRN2 NeuronCores.

Data-parallel over batch: core i computes batch element i end-to-end; no
collectives. Activations are kept "transposed" on device ([channel, seq]) so
per-channel modulation/bias are per-partition scalars; attention scores are
produced directly in [k, q] layout (softmax denominator via a ones-row
appended to V inside the AV matmul). Matmuls run bf16 with fp32 PSUM
accumulation; the residual stream stays fp32.

v2 structural changes vs baseline:
- adaLN mods computed once (was duplicated), in 256-col chunks interleaved
  with the qkv phase; b_ada folded in after the DRAM-transpose readback.
- q AND k inverse-rms folded into qhat/khat via E2-broadcast matmuls
  (removes the kss DRAM roundtrip; exp scale becomes the constant 1/8).
- Scores for a head pair issued back-to-back into one [128,2,512] PSUM tile
  (auto tile_position row packing -> ~2x score throughput), exp over the
  whole [128,1024] in one ACT instruction.
- proj/w3 loops are qch-outer so rms/modulate/w12/output phases pipeline
  per sequence half; attention groups qch-outer so proj fills PE while the
  second half's softmax runs.
- All weight DMAs read host-repacked per-tile-contiguous blocks.
- Engine rebalance: squares/copies/bias-adds on ACT, PSUM->SBUF transposeout
  copies as single strided instructions.
"""

import sys

sys.path.insert(0, "/opt/trn_rl_repo")

import numpy as np
import ml_dtypes

import concourse.bacc as bacc
import concourse.bass as bass
import concourse.mybir as mybir
from concourse.tile import TileContext
from concourse.bass_utils import run_bass_kernel_spmd

F32 = mybir.dt.float32
BF16 = mybir.dt.bfloat16
FP8 = mybir.dt.float8e4
U32 = mybir.dt.uint32
DR = mybir.MatmulPerfMode.DoubleRow
AF = mybir.ActivationFunctionType
ALU = mybir.AluOpType

B, S, D, H = 8, 1024, 1024, 16
HD = D // H  # 64
INNER = 2730
INNER_P = 2816  # 22*128
P = 128
NT = 8
NKT12 = INNER_P // P  # 22
EPS = 1e-6
NADA = 24  # ada chunks of 256 cols

_CACHE = {}


def _to_pmaj(v):
    return np.ascontiguousarray(v.reshape(-1, P).T)


def _rope_perm():
    ev = np.arange(0, HD, 2)
    od = np.arange(1, HD, 2)
    perm = np.concatenate([ev, od])
    partner = np.concatenate([od, ev])
    return perm, partner


def _pack_rows(w):
    """[K, C] -> per 128-col tile contiguous blocks: out[m*128+p, kt*128+c]
    = w[kt*128+p, m*128+c]; result 2D [n_m*128, K/128*128]."""
    K, C = w.shape
    nk = K // P
    nm = C // P
    out = np.zeros((nm * P, nk * P), np.float32)
    for m in range(nm):
        blk = w[:, m * P : (m + 1) * P]  # [K, 128]
        # [nk, 128p, 128c] -> [128p, nk, 128c]
        out[m * P : (m + 1) * P, :] = (
            blk.reshape(nk, P, P).transpose(1, 0, 2).reshape(P, nk * P)
        )
    return out


def _pack_rows_w(w, colw):
    """Like _pack_rows but with arbitrary col tile width colw."""
    K, C = w.shape
    nk = K // P
    nm = C // colw
    out = np.zeros((nm * P, nk * colw), np.float32)
    for m in range(nm):
        blk = w[:, m * colw : (m + 1) * colw]
        out[m * P : (m + 1) * P, :] = (
            blk.reshape(nk, P, colw).transpose(1, 0, 2).reshape(P, nk * colw)
        )
    return out


def _prep_weights(inp):
    """Host-side layout/dtype prep (reordering/padding only, no math)."""
    perm, partner = _rope_perm()
    chperm = (np.arange(D).reshape(H, HD)[:, perm]).reshape(-1)

    w_qkv, b_qkv = inp["w_qkv"], inp["b_qkv"]
    wq = w_qkv[:, 0:D][:, chperm]
    wk = w_qkv[:, D : 2 * D][:, chperm]
    wv = w_qkv[:, 2 * D :]
    bq = b_qkv[0:D][chperm]
    bk = b_qkv[D : 2 * D][chperm]
    bv = b_qkv[2 * D :]
    wv_ext = np.zeros((D, H * 65), np.float32)
    bv_ext = np.zeros((H * 65,), np.float32)
    for h in range(H):
        wv_ext[:, h * 65 : h * 65 + 64] = wv[:, h * 64 : (h + 1) * 64]
        bv_ext[h * 65 : h * 65 + 64] = bv[h * 64 : (h + 1) * 64]
        bv_ext[h * 65 + 64] = 1.0
    wqk_cat = np.concatenate([wq, wk], axis=1)  # [D, 2048]

    w12, b12 = inp["w12"], inp["b12"]
    w12p = np.zeros((D, 2 * INNER_P), np.float32)
    b12p = np.zeros((2 * INNER_P,), np.float32)
    w12p[:, :INNER] = w12[:, :INNER]
    w12p[:, INNER_P : INNER_P + INNER] = w12[:, INNER:]
    b12p[:INNER] = b12[:INNER]
    b12p[INNER_P : INNER_P + INNER] = b12[INNER:]
    w3p = np.zeros((INNER_P, D), np.float32)
    w3p[:INNER] = inp["w3"]
    # fp8 weights are stored pre-scaled by 64 (most mass would be denormal
    # in e4m3 otherwise); compensated by scale=1/64 on the consumer side
    w12p *= 64.0
    w3p *= 64.0

    # rope tiles [128, S]: two stacked 64-row head-local blocks
    sign = np.where(np.arange(HD) < HD // 2, -1.0, 1.0).astype(np.float32)
    cos, sin = inp["rope_cos"], inp["rope_sin"]

    def rope_tiles(scale_vec):
        c64 = cos[:, perm].T * scale_vec[perm][:, None]
        s64 = (sin[:, perm].T * sign[:, None]) * scale_vec[partner][:, None]
        return (
            np.concatenate([c64, c64], 0).astype(np.float32),
            np.concatenate([s64, s64], 0).astype(np.float32),
        )

    cq, sq = rope_tiles(inp["qn_scale"])
    ck, sk = rope_tiles(inp["kn_scale"])

    E2 = np.zeros((2, P), np.float32)
    E2[0, 0:64] = 1.0
    E2[1, 64:128] = 1.0
    e65 = np.zeros((65, 64), np.float32)
    e65[64, :] = 1.0
    bo2 = np.zeros((P, 2), np.float32)
    bo2[0:64, 0] = 1.0
    bo2[64:128, 1] = 1.0

    bqk_T = np.stack(
        [bq.reshape(NT, P)[m] for m in range(NT)]
        + [bk.reshape(NT, P)[m] for m in range(NT)],
        axis=1,
    )

    return {
        "wqkv_t": _pack_rows(wqk_cat),          # [2048, 1024]
        "wv_t": _pack_rows_w(wv_ext, 260),       # [512, 2080]
        "wproj_t": _pack_rows(inp["w_proj"]),    # [1024, 1024]
        "w12_t": _pack_rows(w12p),               # [5632, 1024]
        "w3_t": _pack_rows(w3p),                 # [1024, 2816]
        "wada_t": _pack_rows_w(inp["w_ada"], 256),  # [3072, 2048]
        "bqk_T": bqk_T, "bv_ext": bv_ext[None, :],
        "b12T": _to_pmaj(b12p), "bprojT": _to_pmaj(inp["b_proj"]),
        "b3T": _to_pmaj(inp["b3"]), "n1T": _to_pmaj(inp["norm1_scale"]),
        "n2T": _to_pmaj(inp["norm2_scale"]), "b_adaT": _to_pmaj(inp["b_ada"]),
        "E2": E2, "e65": e65, "bo2": bo2, "ones1": np.ones((1, P), np.float32),
        "ident": np.eye(P, dtype=np.float32),
        "cos2q": cq, "sin2q": sq, "cos2k": ck, "sin2k": sk,
    }


BF16_NAMES = {
    "wqkv_t", "wv_t", "wproj_t", "wada_t", "bv_ext", "E2",
    "e65", "bo2", "ones1", "cos2q", "sin2q", "cos2k", "sin2k",
}
FP8_NAMES = {"w12_t", "w3_t"}

# rsqrt-via-exp bit trick constants: for fp32 v with integer bits I,
# log2(v) ~ I/2^23 - (127 - mu); rsqrt(ss/64) = exp(scale*I + bias)
LN2 = float(np.log(2.0))
RSQ_MU = 0.0430
RSQ_SCALE = -LN2 / (2.0**24)
RSQ_BIAS = 0.5 * LN2 * (127.0 + 6.0 - RSQ_MU)


def build_bass():
    nc = bacc.Bacc("TRN2", target_bir_lowering=False, debug=False, num_devices=8)

    def par(name, shape, dt, out=False):
        return nc.declare_dram_parameter(name, list(shape), dt, isOutput=out)

    d = {
        "x": par("x", [S, D], F32),
        "cT": par("cT", [P, NT], F32),
        "wqkv_t": par("wqkv_t", [2 * D, D], BF16),
        "wv_t": par("wv_t", [4 * P, NT * 260], BF16),
        "wproj_t": par("wproj_t", [D, D], BF16),
        "w12_t": par("w12_t", [2 * INNER_P, D], FP8),
        "w3_t": par("w3_t", [D, INNER_P], FP8),
        "wada_t": par("wada_t", [NADA * P, NT * 256], BF16),
        "bqk_T": par("bqk_T", [P, 16], F32),
        "bv_ext": par("bv_ext", [1, H * 65], BF16),
        "b12T": par("b12T", [P, 2 * NKT12], F32),
        "bprojT": par("bprojT", [P, NT], F32),
        "b3T": par("b3T", [P, NT], F32),
        "n1T": par("n1T", [P, NT], F32),
        "n2T": par("n2T", [P, NT], F32),
        "b_adaT": par("b_adaT", [P, 48], F32),
        "E2": par("E2", [2, P], BF16),
        "e65": par("e65", [65, 64], BF16),
        "bo2": par("bo2", [P, 2], BF16),
        "ones1": par("ones1", [1, P], BF16),
        "ident": par("ident", [P, P], F32),
        "cos2q": par("cos2q", [P, S], BF16),
        "sin2q": par("sin2q", [P, S], BF16),
        "cos2k": par("cos2k", [P, S], BF16),
        "sin2k": par("sin2k", [P, S], BF16),
        "out": par("out", [S, D], F32, out=True),
    }
    mods_dram = nc.dram_tensor("mods_scratch", [1, 6 * D], F32)

    with TileContext(nc) as tc:
        _body(nc, tc, d, mods_dram)
    nc.compile()
    return nc


def _body(nc, tc, d, mods_dram):
    from contextlib import ExitStack

    with ExitStack() as ctx:
        const = ctx.enter_context(tc.tile_pool(name="const", bufs=1))
        persist = ctx.enter_context(tc.tile_pool(name="persist", bufs=1))
        small = ctx.enter_context(tc.tile_pool(name="small", bufs=1))
        scratch = ctx.enter_context(tc.tile_pool(name="scratch", bufs=2))
        # single shared [128,512] PSUM rotation for phases B/C (2 banks)
        mmp = ctx.enter_context(tc.tile_pool(name="mmp", bufs=2, space="PSUM"))

        def load_const(key, shape, dt, pool=None):
            t = (pool or const).tile(list(shape), dt, tag=key, name=key + "_sb")
            nc.sync.dma_start(out=t[:], in_=d[key][:])
            return t

        cT = load_const("cT", [P, NT], F32)
        bqkT = load_const("bqk_T", [P, 16], F32)
        bv = load_const("bv_ext", [1, H * 65], BF16)
        b12T = load_const("b12T", [P, 2 * NKT12], F32)
        bprojT = load_const("bprojT", [P, NT], F32)
        b3T = load_const("b3T", [P, NT], F32)
        n1T = load_const("n1T", [P, NT], F32)
        n2T = load_const("n2T", [P, NT], F32)
        badaT = load_const("b_adaT", [P, 48], F32)
        bo2 = load_const("bo2", [P, 2], BF16)
        e65 = load_const("e65", [65, 64], BF16)
        E2 = load_const("E2", [2, P], BF16)
        ones1 = load_const("ones1", [1, P], BF16)
        ident = load_const("ident", [P, P], F32)
        ones128 = const.tile([P, P], BF16, tag="ones128", name="ones128")
        nc.vector.memset(ones128[:], 1.0)
        eps1 = const.tile([P, 1], F32, tag="eps1", name="eps1")
        nc.vector.memset(eps1[:], EPS)
        rsqb = const.tile([P, 1], F32, tag="rsqb", name="rsqb")
        nc.vector.memset(rsqb[:], RSQ_BIAS)
        rsqb2 = const.tile([P, 1], F32, tag="rsqb2", name="rsqb2")
        nc.vector.memset(rsqb2[:], 0.5 * LN2 * (127.0 + 10.0 - RSQ_MU))

        # residual stream (fp32) + per-token inverse rms, both persistent
        xT = persist.tile([P, NT, S], F32, tag="xT", name="xT")
        invb = persist.tile([P, S], F32, tag="invb", name="invb")
        h2T = persist.tile([P, NT, S], FP8, tag="h2T", name="h2T")

        def rms_half(half, bitexp=False):
            c0 = half * 512
            ms = mmp.tile([P, 512], F32, tag="mm", name="ms")
            for dt in range(NT):
                sq = scratch.tile([P, 512], BF16, tag="sq", name="sq")
                nc.scalar.activation(sq[:], xT[:, dt, c0 : c0 + 512], AF.Square)
                nc.tensor.matmul(
                    ms[:], ones128[:], sq[:], start=(dt == 0), stop=(dt == NT - 1)
                )
            rmsv = scratch.tile([P, 512], F32, tag="rmsv", name="rmsv")
            if bitexp:
                # rsqrt(ms/1024) via exponent-bit log approx (stays in exp set)
                nc.vector.tensor_copy(rmsv[:], ms[:].bitcast(U32))
                nc.scalar.activation(
                    invb[:, c0 : c0 + 512], rmsv[:], AF.Exp,
                    bias=rsqb2[:], scale=RSQ_SCALE,
                )
            else:
                nc.scalar.activation(
                    rmsv[:], ms[:], AF.Sqrt, bias=eps1[:], scale=1.0 / D
                )
                nc.vector.reciprocal_approx_fast(invb[:, c0 : c0 + 512], rmsv[:])

        def modulate_half(dstT, aa, sh, half):
            c0 = half * 512
            for dt in range(NT):
                tmp = scratch.tile([P, 512], BF16, tag="modtmp", name="modtmp")
                nc.vector.tensor_mul(
                    tmp[:], xT[:, dt, c0 : c0 + 512], invb[:, c0 : c0 + 512]
                )
                nc.vector.tensor_scalar(
                    dstT[:, dt, c0 : c0 + 512], tmp[:],
                    aa[:, dt : dt + 1], sh[:, dt : dt + 1],
                    op0=ALU.mult, op1=ALU.add,
                )

        # ===== silu(c) (tiny) =====
        cT_silu = small.tile([P, NT], F32, name="cT_silu")
        nc.scalar.activation(cT_silu[:], cT[:], AF.Silu)
        cs_bf = small.tile([P, NT], BF16, name="cs_bf")
        nc.vector.tensor_copy(cs_bf[:], cT_silu[:])

        # ===== attention superblock =====
        with ExitStack() as actx:
            ho = actx.enter_context(tc.tile_pool(name="ho", bufs=1))
            hT = ho.tile([P, NT, S], BF16, tag="hT", name="hT")
            ohat = ho.tile([P, NT, S], BF16, tag="ohat", name="ohat")
            qhat = ho.tile([P, NT, S], BF16, tag="qhat", name="qhat")
            khat = ho.tile([P, NT, S], BF16, tag="khat", name="khat")
            v_sb = ho.tile([P, NT, H * 65], BF16, tag="v", name="v_sb")

            wada_pool = actx.enter_context(tc.tile_pool(name="wada_pool", bufs=2))
            ada_sc = actx.enter_context(tc.tile_pool(name="ada_sc", bufs=2))

            def ada_chunk(n):
                wt = wada_pool.tile([P, NT, 256], BF16, tag="wada", name="wada_t")
                nc.sync.dma_start(
                    out=wt[:],
                    in_=d["wada_t"][n * P : (n + 1) * P, :].rearrange(
                        "p (kt c) -> p kt c", c=256
                    ),
                )
                ps = mmp.tile([P, 512], F32, tag="mm", name="ps_ada")
                for kt in range(NT):
                    nc.tensor.matmul(
                        ps[0:1, 0:256], cs_bf[:, kt : kt + 1], wt[:, kt, :],
                        start=(kt == 0), stop=(kt == NT - 1),
                    )
                mch = ada_sc.tile([1, 256], F32, tag="mch", name="mch")
                nc.scalar.activation(mch[:], ps[0:1, 0:256], AF.Copy)
                nc.sync.dma_start(
                    out=mods_dram[:, n * 256 : (n + 1) * 256], in_=mch[:]
                )

            # mods chunks 0..7 (sh_msa, sc_msa) first — they gate modulate
            for n in range(8):
                ada_chunk(n)

            # ---- Phase B: load x, transpose to [ch, seq] ----
            with tc.tile_pool(name="xin_pool", bufs=3) as xin_pool, tc.tile_pool(
                name="bpsum", bufs=2, space="PSUM"
            ) as bpsum:
                for st in range(NT):
                    xin = xin_pool.tile([P, D], F32, tag="xin", name="xin")
                    nc.sync.dma_start(out=xin[:], in_=d["x"][st * P : (st + 1) * P, :])
                    for g4 in range(2):
                        pt = bpsum.tile([P, 512], F32, tag="bp", name="pt")
                        for j in range(4):
                            dt = g4 * 4 + j
                            nc.tensor.transpose(
                                pt[:, j * P : (j + 1) * P],
                                xin[:, dt * P : (dt + 1) * P],
                                ident[:],
                            )
                        nc.scalar.activation(
                            xT[:, g4 * 4 : g4 * 4 + 4, st * P : (st + 1) * P],
                            pt[:].rearrange("p (j c) -> p j c", c=P), AF.Copy,
                        )
                    if st == 3:
                        rms_half(0)
                    if st == 7:
                        rms_half(1)

            # early mods readback (chunks 0..7 = sh_msa | sc_msa)
            mT16r = small.tile([P, 16], F32, name="mT16r")
            nc.sync.dma_start(
                out=mT16r[:],
                in_=mods_dram.ap()[0, 0:2048].rearrange("(t p) -> p t", p=P),
            )
            mT16 = small.tile([P, 16], F32, name="mT16")
            nc.vector.tensor_add(mT16[:], mT16r[:], badaT[:, 0:16])
            a1 = small.tile([P, NT], F32, name="a1")
            nc.vector.tensor_scalar_add(a1[:], mT16[:, 8:16], 1.0)
            nc.vector.tensor_mul(a1[:], a1[:], n1T[:])
            sh1 = mT16[:, 0:8]
            modulate_half(hT, a1, sh1, 0)
            modulate_half(hT, a1, sh1, 1)

            # ---- Phase C+D: qkv/rope/v interleaved with attention ----
            with ExitStack() as cctx:
                ropec = cctx.enter_context(tc.tile_pool(name="ropec", bufs=1))
                qkn = cctx.enter_context(tc.tile_pool(name="qkn", bufs=1))
                wqk_pool = cctx.enter_context(tc.tile_pool(name="wqk_pool", bufs=3))
                wv_pool = cctx.enter_context(tc.tile_pool(name="wv_pool", bufs=2))
                rope_sc = cctx.enter_context(tc.tile_pool(name="rope_sc", bufs=2))
                avp = cctx.enter_context(
                    tc.tile_pool(name="avp", bufs=1, space="PSUM")
                )
                ptp = cctx.enter_context(tc.tile_pool(name="ptp", bufs=3))
                att_sc = cctx.enter_context(tc.tile_pool(name="att_sc", bufs=1))
                wproj_pool = cctx.enter_context(
                    tc.tile_pool(name="wproj_pool", bufs=2)
                )
                c_only = cctx.enter_context(ExitStack())
                cpsum = c_only.enter_context(
                    tc.tile_pool(name="cpsum", bufs=2, space="PSUM")
                )
                scp = c_only.enter_context(
                    tc.tile_pool(name="scp", bufs=2, space="PSUM")
                )

                cos2q = load_const("cos2q", [P, S], BF16, pool=ropec)
                sin2q = load_const("sin2q", [P, S], BF16, pool=ropec)
                cos2k = load_const("cos2k", [P, S], BF16, pool=ropec)
                sin2k = load_const("sin2k", [P, S], BF16, pool=ropec)

                def qkv_mtile(m):
                    isq = m < NT
                    mk = m % NT
                    wt = wqk_pool.tile([P, NT, P], BF16, tag="wqk", name="wqk_t")
                    nc.sync.dma_start(
                        out=wt[:],
                        in_=d["wqkv_t"][m * P : (m + 1) * P, :].rearrange(
                            "p (kt c) -> p kt c", c=P
                        ),
                    )
                    raw = rope_sc.tile([P, S], BF16, tag="raw", name="raw")
                    for sch in range(2):
                        ps = mmp.tile([P, 512], F32, tag="mm", name="ps_qkv")
                        for kt in range(NT):
                            nc.tensor.matmul(
                                ps[:], wt[:, kt, :],
                                hT[:, kt, sch * 512 : (sch + 1) * 512],
                                start=(kt == 0), stop=(kt == NT - 1),
                            )
                        nc.scalar.activation(
                            raw[:, sch * 512 : (sch + 1) * 512], ps[:],
                            AF.Identity, bias=bqkT[:, m : m + 1],
                        )
                    sqs = rope_sc.tile([P, S], BF16, tag="t1", name="sqs")
                    nc.vector.tensor_mul(sqs[:], raw[:], raw[:])
                    # rsqrt(ss/64) via exponent-bit log approx folded into Exp
                    # (keeps the whole C+D window inside the exp table set)
                    qf = qkn.tile([2, S], F32, tag="qf", name="qf")
                    qbf = qkn.tile([2, S], BF16, tag="qbf", name="qbf")
                    for sch in range(2):
                        ss = cpsum.tile([P, 512], F32, tag="cp", name="ps_ss")
                        nc.tensor.matmul(
                            ss[0:2, :], bo2[:],
                            sqs[:, sch * 512 : (sch + 1) * 512],
                            start=True, stop=True,
                        )
                        nc.vector.tensor_copy(
                            qf[:, sch * 512 : (sch + 1) * 512],
                            ss[0:2, :].bitcast(U32),
                        )
                        nc.scalar.activation(
                            qbf[:, sch * 512 : (sch + 1) * 512],
                            qf[:, sch * 512 : (sch + 1) * 512],
                            AF.Exp, bias=rsqb[0:2, :], scale=RSQ_SCALE,
                        )
                    rot = rope_sc.tile([P, S], BF16, tag="rot", name="rot")
                    for blk in range(4):
                        b0 = blk * 32
                        srcb = b0 + (32 if blk % 2 == 0 else -32)
                        nc.gpsimd.dma_start(
                            out=rot[b0 : b0 + 32, :], in_=raw[srcb : srcb + 32, :]
                        )
                    t1 = rope_sc.tile([P, S], BF16, tag="t1", name="t1")
                    t2 = rope_sc.tile([P, S], BF16, tag="t2", name="t2")
                    nc.vector.tensor_mul(t1[:], raw[:], (cos2q if isq else cos2k)[:])
                    nc.vector.tensor_mul(t2[:], rot[:], (sin2q if isq else sin2k)[:])
                    t12 = rope_sc.tile([P, S], BF16, tag="rot", name="t12")
                    nc.vector.tensor_add(t12[:], t1[:], t2[:])
                    dst = qhat if isq else khat
                    for sch in range(2):
                        pe = cpsum.tile([P, 512], F32, tag="cp", name="ps_e2")
                        nc.tensor.matmul(
                            pe[:], E2[:], qbf[:, sch * 512 : (sch + 1) * 512],
                            start=True, stop=True,
                        )
                        nc.vector.tensor_mul(
                            dst[:, mk, sch * 512 : (sch + 1) * 512],
                            t12[:, sch * 512 : (sch + 1) * 512], pe[:],
                        )

                def v_chunk(c):
                    wt = wv_pool.tile([P, NT, 260], BF16, tag="wv", name="wv_t")
                    nc.sync.dma_start(
                        out=wt[:],
                        in_=d["wv_t"][c * P : (c + 1) * P, :].rearrange(
                            "p (kt c2) -> p kt c2", c2=260
                        ),
                    )
                    c0 = c * 260
                    for st in range(NT):
                        ps = cpsum.tile([P, 512], F32, tag="cp", name="ps_v")
                        for kt in range(NT):
                            nc.tensor.matmul(
                                ps[:, 0:260], hT[:, kt, st * P : (st + 1) * P],
                                wt[:, kt, :], start=(kt == 0), stop=False,
                            )
                        nc.tensor.matmul(
                            ps[:, 0:260], ones1[:], bv[:, c0 : c0 + 260],
                            start=False, stop=True,
                        )
                        nc.vector.tensor_copy(
                            v_sb[:, st, c0 : c0 + 260], ps[:, 0:260]
                        )

                def attn_group(mk, qch, sc_pool, sc_tag, paired):
                    q0 = qch * 512
                    avt = avp.tile([65, 2, 512], F32, tag="av", name="avt")
                    for kt in range(NT):
                        if paired:
                            sct2 = sc_pool.tile(
                                [P, 2, 512], F32, tag=sc_tag, name="sct2"
                            )
                            for hh in range(2):
                                rb = 64 * hh
                                nc.tensor.matmul(
                                    sct2[:, hh, :],
                                    khat[rb : rb + 64, mk, kt * P : (kt + 1) * P],
                                    qhat[rb : rb + 64, mk, q0 : q0 + 512],
                                    start=True, stop=True,
                                )
                            pt2 = ptp.tile([P, 2, 512], BF16, tag="pT", name="pt2")
                            nc.scalar.activation(
                                pt2[:], sct2[:], AF.Exp, scale=0.125
                            )
                            pts = [pt2[:, 0, :], pt2[:, 1, :]]
                        else:
                            pts = []
                            for hh in range(2):
                                rb = 64 * hh
                                sct = sc_pool.tile(
                                    [P, 512], F32, tag=sc_tag, name="sct"
                                )
                                nc.tensor.matmul(
                                    sct[:],
                                    khat[rb : rb + 64, mk, kt * P : (kt + 1) * P],
                                    qhat[rb : rb + 64, mk, q0 : q0 + 512],
                                    start=True, stop=True,
                                )
                                pt = ptp.tile([P, 512], BF16, tag="pT", name="pt")
                                nc.scalar.activation(
                                    pt[:], sct[:], AF.Exp, scale=0.125
                                )
                                pts.append(pt[:])
                        for hh in range(2):
                            h = 2 * mk + hh
                            nc.tensor.matmul(
                                avt[:, hh, :], v_sb[:, kt, h * 65 : h * 65 + 65],
                                pts[hh],
                                start=(kt == 0), stop=(kt == NT - 1),
                            )
                    for hh in range(2):
                        o65b = att_sc.tile([65, 512], BF16, tag="o65b", name="o65b")
                        nc.vector.tensor_copy(o65b[:], avt[:, hh, :])
                        pb = sc_pool.tile([P, 512], F32, tag=sc_tag, name="ps_bc")
                        nc.tensor.matmul(
                            pb[0:64, :], e65[:], o65b[:], start=True, stop=True
                        )
                        rb64 = att_sc.tile([64, 512], F32, tag="rb64", name="rb64")
                        nc.vector.reciprocal_approx_fast(rb64[:], pb[0:64, :])
                        if hh == 0:
                            nc.vector.tensor_mul(
                                ohat[0:64, mk, q0 : q0 + 512],
                                o65b[0:64, :], rb64[:],
                            )
                        else:
                            ob = att_sc.tile([64, 512], BF16, tag="ob", name="ob")
                            nc.vector.tensor_mul(ob[:], o65b[0:64, :], rb64[:])
                            nc.sync.dma_start(
                                out=ohat[64:128, mk, q0 : q0 + 512], in_=ob[:]
                            )

                def proj_qch(qch):
                    for dt in range(NT):
                        wt = wproj_pool.tile(
                            [P, NT, P], BF16, tag="wproj", name="wproj_t"
                        )
                        nc.sync.dma_start(
                            out=wt[:],
                            in_=d["wproj_t"][dt * P : (dt + 1) * P, :].rearrange(
                                "p (kt c) -> p kt c", c=P
                            ),
                        )
                        ps = mmp.tile([P, 512], F32, tag="mm", name="ps_proj")
                        for kt in range(NT):
                            nc.tensor.matmul(
                                ps[:], wt[:, kt, :],
                                ohat[:, kt, qch * 512 : (qch + 1) * 512],
                                start=(kt == 0), stop=(kt == NT - 1),
                            )
                        nc.vector.affine_then_add(
                            xT[:, dt, qch * 512 : (qch + 1) * 512],
                            ps[:], xT[:, dt, qch * 512 : (qch + 1) * 512],
                            scale=g1[:, dt : dt + 1], bias=g1b[:, dt : dt + 1],
                        )

                # interleave: q/k m-tiles, v chunks, ada chunks, attention
                m_order = [0, 8, 1, 9, 2, 10, 3, 11, 4, 12, 5, 13, 6, 14, 7, 15]
                ada_next = 8
                for i, m in enumerate(m_order):
                    qkv_mtile(m)
                    if i in (3, 7, 11, 15):
                        v_chunk(i // 4)
                    if i % 2 == 1 and ada_next < NADA:
                        ada_chunk(ada_next)
                        ada_chunk(ada_next + 1)
                        ada_next += 2
                    if i >= 3 and i % 2 == 1:
                        attn_group((i - 3) // 2, 0, scp, "sc", False)
                attn_group(7, 0, scp, "sc", False)
                c_only.close()
                scp2 = cctx.enter_context(
                    tc.tile_pool(name="scp2", bufs=2, space="PSUM")
                )

                # full mods readback + derived scalars
                mT48r = small.tile([P, 48], F32, name="mT48r")
                nc.sync.dma_start(
                    out=mT48r[:],
                    in_=mods_dram.ap()[0, :].rearrange("(t p) -> p t", p=P),
                )
                mT48 = small.tile([P, 48], F32, name="mT48")
                nc.vector.tensor_add(mT48[:], mT48r[:], badaT[:])
                g1 = mT48[:, 16:24]
                g1b = small.tile([P, NT], F32, name="g1b")
                nc.vector.tensor_mul(g1b[:], g1, bprojT[:])
                a2 = small.tile([P, NT], F32, name="a2")
                nc.vector.tensor_scalar_add(a2[:], mT48[:, 32:40], 1.0)
                nc.vector.tensor_mul(a2[:], a2[:], n2T[:])
                sh2 = mT48[:, 24:32]
                g2 = mT48[:, 40:48]
                g2b3 = small.tile([P, NT], F32, name="g2b3")
                nc.vector.tensor_mul(g2b3[:], g2, b3T[:])
                g2s = small.tile([P, NT], F32, name="g2s")
                nc.scalar.mul(g2s[:], g2, 1.0 / 64.0)

                # second attention wave (qch=1); proj + rms2/mod2 fill PE gaps
                for mk in range(NT):
                    attn_group(mk, 1, scp2, "sc2", True)
                proj_qch(0)
                rms_half(0, bitexp=True)
                modulate_half(h2T, a2, sh2, 0)
                proj_qch(1)
                rms_half(1, bitexp=True)
                modulate_half(h2T, a2, sh2, 1)

        # ===== MLP phase =====
        with ExitStack() as mctx:
            mlp = mctx.enter_context(tc.tile_pool(name="mlp", bufs=1))
            mpsum = mctx.enter_context(
                tc.tile_pool(name="mpsum", bufs=6, space="PSUM")
            )

            gg = mlp.tile([P, NKT12, S], FP8, tag="gg", name="gg")
            w3_sb = mlp.tile([P, NT, NKT12 * P], FP8, tag="w3sb", name="w3_sb")
            nc.sync.dma_start(
                out=w3_sb[:],
                in_=d["w3_t"][:, :].rearrange("(dt p) c -> p dt c", p=P),
            )

            with tc.tile_pool(name="w12_pool", bufs=3) as w12_pool, tc.tile_pool(
                name="mlp_sc", bufs=2
            ) as mlp_sc:
                for j in range(NKT12):
                    outs = []
                    for part in range(2):
                        m = j + part * NKT12
                        wt = w12_pool.tile([P, NT, P], FP8, tag="w12", name="w12_t")
                        nc.sync.dma_start(
                            out=wt[:],
                            in_=d["w12_t"][m * P : (m + 1) * P, :].rearrange(
                                "p (kt c) -> p kt c", c=P
                            ),
                        )
                        o = mlp_sc.tile([P, S], BF16, tag=f"mlp{part}", name=f"mlp{part}")
                        for sch in range(2):
                            ps = mpsum.tile([P, 512], F32, tag="ps", name="ps_mlp")
                            for ktp in range(NT // 2):
                                nc.tensor.matmul(
                                    ps[:], wt[:, 2 * ktp : 2 * ktp + 2, :],
                                    h2T[
                                        :, 2 * ktp : 2 * ktp + 2,
                                        sch * 512 : (sch + 1) * 512,
                                    ],
                                    start=(ktp == 0), stop=(ktp == NT // 2 - 1),
                                    perf_mode=DR,
                                )
                            nc.scalar.activation(
                                o[:, sch * 512 : (sch + 1) * 512], ps[:],
                                AF.Silu if part == 0 else AF.Identity,
                                bias=b12T[:, m : m + 1], scale=1.0 / 64.0,
                            )
                        outs.append(o)
                    nc.vector.tensor_mul(gg[:, j, :], outs[0][:], outs[1][:])

            # w3 + residual 2 (qch-outer) + output transpose per half
            with tc.tile_pool(name="yout", bufs=3) as ypool:
                for qch in range(2):
                    for dt in range(NT):
                        ps = mpsum.tile([P, 512], F32, tag="ps", name="ps_w3")
                        for ktp in range(NKT12 // 2):
                            nc.tensor.matmul(
                                ps[:],
                                w3_sb[
                                    :, dt, 2 * ktp * P : (2 * ktp + 2) * P
                                ].rearrange("p (g c) -> p g c", g=2),
                                gg[
                                    :, 2 * ktp : 2 * ktp + 2,
                                    qch * 512 : (qch + 1) * 512,
                                ],
                                start=(ktp == 0), stop=(ktp == NKT12 // 2 - 1),
                                perf_mode=DR,
                            )
                        nc.vector.affine_then_add(
                            xT[:, dt, qch * 512 : (qch + 1) * 512],
                            ps[:], xT[:, dt, qch * 512 : (qch + 1) * 512],
                            scale=g2s[:, dt : dt + 1], bias=g2b3[:, dt : dt + 1],
                        )
                    for st in range(qch * 4, qch * 4 + 4):
                        y = ypool.tile([P, D], F32, tag="y", name="y")
                        for g4 in range(2):
                            pt = mpsum.tile([P, 512], F32, tag="ps", name="ps_tr2")
                            for j in range(4):
                                dt = g4 * 4 + j
                                nc.tensor.transpose(
                                    pt[:, j * P : (j + 1) * P],
                                    xT[:, dt, st * P : (st + 1) * P],
                                    ident[:],
                                )
                            nc.scalar.activation(
                                y[:, g4 * 512 : (g4 + 1) * 512], pt[:], AF.Copy
                            )
                        nc.sync.dma_start(
                            out=d["out"][st * P : (st + 1) * P, :], in_=y[:]
                        )


def kernel(**inputs):
    inputs = {k: np.asarray(v) for k, v in inputs.items()}
    if "nc" not in _CACHE:
        _CACHE["nc"] = build_bass()
    nc = _CACHE["nc"]

    consts = _prep_weights(inputs)
    base = {}
    for k, v in consts.items():
        if k in BF16_NAMES:
            base[k] = np.ascontiguousarray(v).astype(ml_dtypes.bfloat16)
        elif k in FP8_NAMES:
            base[k] = np.ascontiguousarray(v).astype(ml_dtypes.float8_e4m3fn)
        else:
            base[k] = np.ascontiguousarray(v).astype(np.float32)

    in_maps = []
    for core in range(B):
        m = dict(base)
        m["x"] = np.ascontiguousarray(inputs["x"][core]).astype(np.float32)
        m["cT"] = _to_pmaj(inputs["c"][core]).astype(np.float32)
        in_maps.append(m)

    res = run_bass_kernel_spmd(
        nc, in_maps, core_ids=list(range(B)), **_CACHE.get("run_kwargs", {})
    )
    _CACHE["last_results"] = res
    return np.stack([res.results[i]["out"] for i in range(B)], axis=0)


if __name__ == "__main__":
    build_bass()
    print("built ok")
